# revision 56
# baseline (speedup 1.0000x reference)
"""BitNet SwiGLU MLP kernel for Trainium2, tensor-parallel over 8 NeuronCores.

Sharding (Megatron-style TP over the intermediate dim F):
- Each core holds a 1/8 column-shard of Wg/Wu (fed transposed: [D, FS]) and
  the matching shard of Wd (fed as Wd[:, shard].T = [FS, D]). x is replicated,
  fed both natural-sliced (per-token quant stats, sharded over tokens) and
  fully transposed [D, T] (matmul operand layout).
- bit_linear runs as an exact integer matmul: quantized activations are ints
  in [-128,127] (bf16 lhsT) and ternary weights in {-1,0,1} (fp8e4 rhs, both
  exact, accumulated exactly in fp32 PSUM), dequantized on the output by
  per-token / global scales. clip(round(.)) for activations is exact RNE via
  the fp32 magic-number trick (the clip never binds since |x*scale| <= 127).
- Down-proj + mean-pool is collapsed algebraically:
  mean_{h,d}(hq @ Wdq.T) = 1/(H*D) * sum_f hq[t,f] * S[f],  S = colsum(Wdq)
  so only a per-token weighted row-reduction against S remains.
- h is kept resident in SBUF as fp16 between the main loop and the
  per-token requantization pass; no DRAM roundtrip.
- Head is latency-optimized: weight |.| stats stream dual-engine
  (vector+scalar) as DMA lands, cross-partition sums go through gpsimd (the
  PE never sits on the c1 critical path), the quant-pass re-read is
  prefetched during the c1 AllReduce wait, and the Wd stat/quant work is
  deferred into the early loop groups where engines have slack.
- Per-segment RMS/absmax stats cross cores via ONE AllReduce(max) with
  per-core slots (mask built from a one-hot input); pooled partials are
  AllReduced at the end; every core runs the tiny classifier.
"""
import numpy as np

MAGIC = 12582912.0  # 1.5 * 2^23, fp32 RNE magic
EPS = 1e-6
QEPS = 1e-5


def build(B=8, C=3, H=128, D=2048, F=8192, NCLS=1000, NCORES=8,
          ln_is_ones=True, mock_collectives=False,
          NDUM1=480, NDUM2=55, H_BUFS=20, XQ_BUFS=20, RQ_BUFS=2,
          SAFE_CLIP=False, SAFE_BCAST=True, SAFE_TTR=True):
    """Build + compile the SPMD Bass program. Returns (nc, meta)."""
    import concourse.bacc as bacc
    import concourse.tile as tile
    from concourse import mybir
    from concourse import bass_isa

    f32 = mybir.dt.float32
    bf16 = mybir.dt.bfloat16
    fp16 = mybir.dt.float16
    fp8 = mybir.dt.float8e4
    AX = mybir.AxisListType
    OP = mybir.AluOpType
    AF = mybir.ActivationFunctionType
    RO = bass_isa.ReduceOp
    RG = [list(range(NCORES))]

    assert H == 128
    T = B * C * H
    TT = T // 128               # token tiles (== B*C) = 24
    TS = T // NCORES            # tokens per core for x stats
    TST = TS // 128
    FS = F // NCORES            # f-shard width = 1024
    DT = D // 128               # contraction tiles = 16
    NF = 512
    FH = FS // NF               # = 2
    WB = 2                      # d-tiles per weight/x DMA slab

    # token-tile groups (last ones smaller to tighten the tail)
    GROUPS = [(0, 4), (4, 4), (8, 4), (12, 4), (16, 4), (20, 2), (22, 2)]
    assert sum(g[1] for g in GROUPS) == TT
    SEGB = [0, 4, 8, 12, 16, 20, 23, 24]
    NSEG = len(SEGB) - 1
    def group_of(t):
        for gi, (t0, gsz) in enumerate(GROUPS):
            if t0 <= t < t0 + gsz:
                return gi
        raise AssertionError
    # post-stats for segment s run AFTER group gpost's tiles (so an engine
    # FIFO wait on the segment's collective can never block later tiles'
    # compute that feeds later collectives). Early segments get extra slack
    # because Sh_bcast (built from the in-loop Wd quant) lands around g3.
    # Sh_bcast (from the in-loop Wd quant chain) lands at the end of group
    # 4, so the early segments all post right after it; h tiles are fully
    # SBUF-resident so late posts never throttle the loop.
    POST_AT = {4: [0, 1, 2], 5: [3, 4], 6: [5]}
    POST_TAIL = [6]

    nc = bacc.Bacc("TRN2", target_bir_lowering=False, debug=False,
                   num_devices=1 if mock_collectives else NCORES)

    def collective(kind, op, in_ap, out_ap):
        if NCORES == 1 or mock_collectives:
            n = out_ap.size() // in_ap.size()
            flat = out_ap.rearrange("a b -> (a b)")
            for r in range(n):
                nc.sync.dma_start(
                    flat[r * in_ap.size():(r + 1) * in_ap.size()], in_ap)
        else:
            nc.gpsimd.collective_compute(kind, op, replica_groups=RG,
                                         ins=[in_ap.opt()], outs=[out_ap.opt()])

    xs_t = nc.dram_tensor("xs", [TS, D], f32, kind="ExternalInput")
    xT_t = nc.dram_tensor("xT", [D, T], f32, kind="ExternalInput")
    wgT_t = nc.dram_tensor("wgT", [D, FS], f32, kind="ExternalInput")
    wuT_t = nc.dram_tensor("wuT", [D, FS], f32, kind="ExternalInput")
    wdT_t = nc.dram_tensor("wdT", [FS, D], f32, kind="ExternalInput")
    lnw_t = nc.dram_tensor("lnw", [1, FS], f32, kind="ExternalInput")
    clsWT_t = nc.dram_tensor("clsWT", [C, NCLS], f32, kind="ExternalInput")
    clsb_t = nc.dram_tensor("clsb", [1, NCLS], f32, kind="ExternalInput")
    cmask_t = nc.dram_tensor("cmask", [1, NCORES], f32, kind="ExternalInput")
    out_t = nc.dram_tensor("out", [B, NCLS], f32, kind="ExternalOutput")

    def r128(ap):
        # [1, n*128] dram view -> [128, n] (partition = fast axis)
        return ap.rearrange("o (i p) -> (o p) i", p=128)

    with tile.TileContext(nc) as tc:
        import contextlib
        with contextlib.ExitStack() as st:
            dram = st.enter_context(tc.tile_pool(name="dram", bufs=1, space="DRAM"))
            sbC = st.enter_context(tc.tile_pool(name="sbC", bufs=1))
            sbS8 = st.enter_context(tc.tile_pool(name="sbS8", bufs=2))
            sbT1 = st.enter_context(tc.tile_pool(name="sbT1", bufs=2))
            sbS2 = st.enter_context(tc.tile_pool(name="sbS2", bufs=3))
            sbUG = st.enter_context(tc.tile_pool(name="sbUG", bufs=2))
            sbCol = st.enter_context(tc.tile_pool(name="sbCol", bufs=4))
            sbH = st.enter_context(tc.tile_pool(name="sbH", bufs=H_BUFS))
            sbXQ = st.enter_context(tc.tile_pool(name="sbXQ", bufs=XQ_BUFS))
            sbWQ = st.enter_context(tc.tile_pool(name="sbWQ", bufs=2 * DT))

            sc_in = dram.tile([1, TS], f32)
            sc_out = dram.tile([1, T], f32)
            c1_in = dram.tile([1, 8], f32)
            c1_out = dram.tile([1, 8], f32)
            c2_in = dram.tile([1, 8], f32)
            c2_out = dram.tile([1, 8], f32)
            srow_dram = dram.tile([1, FS], f32)
            srow2_dram = dram.tile([1, T], f32)
            dum_dram = dram.tile([1, 8], f32)
            # slotted stats exchange: one AllReduce(max) per segment over
            # [256, SEG*8]; core k's ssq/am partials sit in slot k of the
            # innermost axis (all other slots zero, and partials are >= 0,
            # so max == gather). Local free-axis reduce then combines slots.
            st_in = [dram.tile([256, (SEGB[s + 1] - SEGB[s]) * NCORES], f32,
                               name=f"st_in{s}") for s in range(NSEG)]
            st_out = [dram.tile([256, (SEGB[s + 1] - SEGB[s]) * NCORES], f32,
                                name=f"st_out{s}", addr_space="Shared")
                      for s in range(NSEG)]
            pl_in = dram.tile([1, TT], f32)
            pl_out = dram.tile([1, TT], f32, addr_space="Shared")

            ones1 = sbC.tile([1, 128], f32)
            nc.vector.memset(ones1[:], 1.0)
            ones_col = sbC.tile([128, 1], f32)
            nc.vector.memset(ones_col[:], 1.0)
            negmagic = sbC.tile([128, 1], f32)
            nc.vector.memset(negmagic[:], -MAGIC)
            zeros_bf = sbC.tile([128, NF], bf16)
            nc.vector.memset(zeros_bf[:], 0.0)

            wacc = sbC.tile([128, 48], f32)
            mask_bcast = sbC.tile([128, NCORES], f32)
            sc_cols = sbC.tile([128, TST], f32)
            c1_sb = sbC.tile([1, 8], f32)
            c2_sb = sbC.tile([1, 8], f32)
            m_w_col = sbC.tile([128, 2], f32)
            s_w_col = sbC.tile([128, 2], f32)
            m_wd_col = sbC.tile([128, 1], f32)
            s_wd_col = sbC.tile([128, 1], f32)
            S_bcast = sbC.tile([128, T], f32)
            Sh_bcast = sbC.tile([128, FS], f32)
            DEQG = sbC.tile([128, TT], f32)
            DEQU = sbC.tile([128, TT], f32)
            ssq_cols = sbC.tile([128, TT], f32)
            am_cols = sbC.tile([128, TT], f32)
            Q_cols = sbC.tile([128, TT], f32)
            Ssh_cols = sbC.tile([128, FS // 128], f32)
            if not ln_is_ones:
                Ln_bcast = sbC.tile([128, FS], f32)

            ht_tiles = [None] * TT

            def bcast_row(out_cols, in_row, n):
                # out_cols [128, n] <- broadcast of in_row [1, n]
                if SAFE_BCAST:
                    pad = sbCol.tile([128, max(n, 1)], f32, tag="bc",
                                     bufs=2, name="bcpad")
                    nc.vector.memset(pad[:, 0:n], 0.0)
                    nc.vector.tensor_copy(pad[0:1, 0:n], in_row[0:1, 0:n])
                    nc.gpsimd.partition_all_reduce(out_cols[:, 0:n],
                                                   pad[:, 0:n], channels=128,
                                                   reduce_op=RO.add)
                else:
                    nc.gpsimd.partition_broadcast(out_cols[:, 0:n],
                                                  in_row[0:1, 0:n],
                                                  channels=128)

            def clip_step(t1):
                # clamp t1 (rounded magic form) to [MAGIC-1, MAGIC+1]
                if SAFE_CLIP:
                    t2 = sbT1.tile([128, FS], f32, tag="T2", bufs=2)
                    nc.vector.tensor_scalar(out=t2[:], in0=t1[:],
                                            scalar1=MAGIC - 1.0,
                                            scalar2=MAGIC + 1.0,
                                            op0=OP.max, op1=OP.min)
                    return t2
                nc.vector.tensor_scalar(out=t1[:], in0=t1[:],
                                        scalar1=MAGIC - 1.0,
                                        scalar2=MAGIC + 1.0,
                                        op0=OP.max, op1=OP.min)
                return t1

            def mult_rowsum(in0, in1, accum, init):
                # accum [128,1] = init + sum_cols(in0 * in1)
                if SAFE_TTR:
                    jk = sbUG.tile([128, FS], bf16, tag="jkf", bufs=2,
                                   name="jkf")
                    nc.vector.tensor_tensor(out=jk[:], in0=in0, in1=in1,
                                            op=OP.mult)
                    if init is None:
                        nc.vector.tensor_reduce(out=accum[:], in_=jk[:],
                                                axis=AX.X, op=OP.add)
                    else:
                        part = sbCol.tile([128, 1], f32, tag="mrs",
                                          name="mrs")
                        nc.vector.tensor_reduce(out=part[:], in_=jk[:],
                                                axis=AX.X, op=OP.add)
                        nc.vector.tensor_tensor(out=accum[:], in0=part[:],
                                                in1=init[:], op=OP.add)
                else:
                    jk = sbUG.tile([128, FS], bf16, tag="jk", bufs=2,
                                   name="jk")
                    nc.vector.tensor_tensor_reduce(
                        out=jk[:], in0=in0, in1=in1, scale=1.0,
                        scalar=(0.0 if init is None else init[:]),
                        op0=OP.mult, op1=OP.add, accum_out=accum[:])

            def abs_stat(wt, wcol, on_scalar):
                # wcol [128,1] = sum_cols |wt|, on either engine
                if on_scalar:
                    # the elementwise output is never read (fp8 scratch);
                    # only the fp32 row-accumulator matters
                    wsc = sbT1.tile([128, wt.free_size()], fp8, tag="wsc",
                                    bufs=2, name="wsc")
                    nc.scalar.activation(out=wsc[:], in_=wt, func=AF.Abs,
                                         accum_out=wcol)
                else:
                    nc.vector.tensor_reduce(out=wcol, in_=wt, axis=AX.X,
                                            op=OP.add,
                                            apply_absolute_value=True)

            # emit_xq defined up-front: the head emits group 0 itself
            def emit_xq(gi, xq_slabs):
                t0, gsz = GROUPS[gi]
                gw = gsz * 128
                tc0 = t0 * 128
                for d0 in range(0, DT, WB):
                    xsl = sbXQ.tile([128, WB * 512], f32, tag="xsl", bufs=2)
                    nc.sync.dma_start(
                        xsl[:, 0:WB * gw],
                        xT_t.ap()[d0 * 128:(d0 + WB) * 128, tc0:tc0 + gw]
                        .rearrange("(b p) c -> p b c", p=128))
                    for b in range(WB):
                        xp = sbXQ.tile([128, 512], f32, tag="xp", bufs=2)
                        nc.vector.tensor_tensor(
                            out=xp[:, 0:gw], in0=xsl[:, b * gw:(b + 1) * gw],
                            in1=S_bcast[:, tc0:tc0 + gw], op=OP.mult)
                        xq = sbXQ.tile([128, 512], bf16, tag="xq")
                        nc.vector.tensor_scalar(out=xq[:, 0:gw],
                                                in0=xp[:, 0:gw],
                                                scalar1=MAGIC, scalar2=MAGIC,
                                                op0=OP.add, op1=OP.subtract)
                        xq_slabs[d0 + b] = xq
                return xq_slabs

            # ================= Head =================
            wd_tiles = [None] * (2 * (FS // 128))
            with tc.tile_pool(name="psH", bufs=1, space="PSUM") as psH:
                # dummy matmuls keep the PE p-state up until real MMs arrive
                if NDUM1 > 0:
                    dum = psH.tile([128, NF], f32, tag="dum")
                    for k in range(NDUM1):
                        nc.tensor.matmul(dum[:], zeros_bf[:, 0:128],
                                         zeros_bf[:], start=(k == 0),
                                         stop=(k == NDUM1 - 1))

                # x per-token clipped absmax -> sc_in -> AllGather (first
                # collective; also warms the CC stream behind the barrier).
                # Halved DMAs ride both hardware DGE queues concurrently.
                for i in range(TST):
                    hs = []
                    for h2 in range(2):
                        xt = sbS8.tile([128, FS], f32, bufs=4,
                                       tag=("Sg" if h2 == 0 else "Su"))
                        nc.gpsimd.dma_start(
                            xt[:], xs_t.ap()[i * 128:(i + 1) * 128,
                                             h2 * FS:(h2 + 1) * FS])
                        hs.append(xt)
                    am = sbCol.tile([128, 2], f32, tag="am")
                    for h2 in range(2):
                        nc.vector.tensor_reduce(out=am[:, h2:h2 + 1],
                                                in_=hs[h2][:], axis=AX.X,
                                                op=OP.max,
                                                apply_absolute_value=True)
                    am1 = sbCol.tile([128, 1], f32, tag="am1")
                    nc.vector.tensor_reduce(out=am1[:], in_=am[:], axis=AX.X,
                                            op=OP.max)
                    nc.vector.tensor_scalar(out=sc_cols[:, i:i + 1], in0=am1[:],
                                            scalar1=QEPS, scalar2=None,
                                            op0=OP.max)
                nc.sync.dma_start(r128(sc_in[:]), sc_cols[:])
                collective("AllGather", OP.bypass, sc_in[:], sc_out[:])

                # Wg/Wu |.| sums. Two hardware DGE queues run concurrently:
                # g slabs load via SP (reduce on vector), u slabs via the
                # Activation HWDGE (reduce on scalar). Stats lag their slab
                # by one iteration so neither engine FIFO blocks the other
                # queue's triggers.
                stat_q = []
                stat_wts = {}
                for d in range(DT):
                    wts = []
                    for j, ten in enumerate((wgT_t, wuT_t)):
                        eng = nc.sync if j == 0 else nc.scalar
                        wt = sbS8.tile([128, FS], f32, bufs=4,
                                       tag=("Sg" if j == 0 else "Su"))
                        eng.dma_start(wt[:],
                                      ten.ap()[d * 128:(d + 1) * 128, :])
                        wts.append(wt)
                    stat_wts[d] = wts
                    stat_q.append((d, wts))
                    if len(stat_q) > 1:
                        pi, pw = stat_q.pop(0)
                        for j in range(2):
                            abs_stat(pw[j][:], wacc[:, j * 16 + pi:j * 16 + pi + 1],
                                     on_scalar=False)
                for pi, pw in stat_q:
                    for j in range(2):
                        abs_stat(pw[j][:], wacc[:, j * 16 + pi:j * 16 + pi + 1],
                                 on_scalar=False)
                # cross-partition totals via gpsimd (PE stays off this path)
                colgu = sbCol.tile([128, 2], f32, tag="cgu")
                nc.vector.tensor_reduce(out=colgu[:, 0:1], in_=wacc[:, 0:16],
                                        axis=AX.X, op=OP.add)
                nc.vector.tensor_reduce(out=colgu[:, 1:2], in_=wacc[:, 16:32],
                                        axis=AX.X, op=OP.add)
                c1gu = sbCol.tile([128, 2], f32, tag="c1gu")
                nc.gpsimd.partition_all_reduce(c1gu[:], colgu[:], channels=128,
                                               reduce_op=RO.add)
                nc.vector.memset(c1_sb[:], 0.0)
                nc.vector.tensor_copy(c1_sb[:, 0:2], c1gu[0:1, 0:2])
                nc.sync.dma_start(c1_in[:], c1_sb[:])
                collective("AllReduce", OP.add, c1_in[:], c1_out[:])

                # S_bcast = 127 / absmax: the reciprocal runs on the
                # compact [128, TT] gathered form (DVE reciprocal is slow),
                # then a DRAM bounce re-rows it and the PE broadcasts it
                # down partitions. Emitted before emit_xq (vector FIFO
                # ordering). Numerically identical to reciprocal-then-scale
                # on the broadcast form.
                yraw = sbC.tile([128, TT], f32)
                nc.gpsimd.dma_start(yraw[:], r128(sc_out[:]))
                r24 = sbC.tile([128, TT], f32)
                nc.vector.reciprocal(r24[:], yraw[:])
                s24 = sbC.tile([128, TT], f32)
                nc.vector.tensor_scalar(out=s24[:], in0=r24[:],
                                        scalar1=127.0, scalar2=None,
                                        op0=OP.mult)
                nc.sync.dma_start(r128(srow2_dram[:]), s24[:])
                for j in range(0, T, 512):
                    scr = sbS2.tile([1, 512], f32, tag="scr")
                    nc.gpsimd.dma_start(scr[:], srow2_dram[0:1, j:j + 512])
                    pb = psH.tile([128, 512], f32, tag="pb", bufs=2)
                    nc.tensor.matmul(pb[:], ones1[:], scr[:],
                                     start=True, stop=True)
                    nc.scalar.activation(out=S_bcast[:, j:j + 512],
                                         in_=pb[:], func=AF.Copy)

                # group-0 xq: xT DMAs queue right behind the stat reads
                cur_xq = emit_xq(0, [None] * DT)

                # ---- derived scalars from c1 (g,u) ----
                # (these DMAs are emitted BEFORE the rq stream below so no
                # rq slab -- whose buffer release depends on them -- can sit
                # ahead of them in a DMA queue: that would deadlock)
                c1g = sbC.tile([1, 8], f32)
                nc.gpsimd.dma_start(c1g[:], c1_out[:])
                m_w = sbC.tile([1, 2], f32)
                nc.vector.tensor_scalar(out=m_w[:], in0=c1g[:, 0:2],
                                        scalar1=1.0 / (float(F) * D),
                                        scalar2=QEPS, op0=OP.mult, op1=OP.max)
                s_w = sbC.tile([1, 2], f32)
                nc.vector.reciprocal(s_w[:], m_w[:])
                bcast_row(m_w_col, m_w, 2)
                bcast_row(s_w_col, s_w, 2)

                nc.vector.tensor_scalar(out=DEQG[:], in0=yraw[:],
                                        scalar1=m_w_col[:, 0:1],
                                        scalar2=1.0 / 127.0,
                                        op0=OP.mult, op1=OP.mult)
                nc.vector.tensor_scalar(out=DEQU[:], in0=yraw[:],
                                        scalar1=m_w_col[:, 1:2],
                                        scalar2=1.0 / 127.0,
                                        op0=OP.mult, op1=OP.mult)

                # per-core one-hot slot mask (c1-wait window)
                cm_row = sbC.tile([1, NCORES], f32)
                nc.sync.dma_start(cm_row[:], cmask_t.ap())
                bcast_row(mask_bcast, cm_row, NCORES)

                # second dummy batch bridges the gap to the first real MMs
                if NDUM1 > 0 and NDUM2 > 0:
                    dum2 = psH.tile([128, NF], f32, tag="dum2")
                    for k in range(NDUM2):
                        nc.tensor.matmul(dum2[:], zeros_bf[:, 0:128],
                                         zeros_bf[:], start=(k == 0),
                                         stop=(k == NDUM2 - 1))
                    dsc = sbC.tile([1, 8], f32)
                    nc.vector.memset(dsc[:], 0.0)
                    nc.vector.tensor_copy(dsc[:, 0:1], dum[0:1, 0:1])
                    nc.vector.tensor_copy(dsc[:, 1:2], dum2[0:1, 0:1])
                    nc.sync.dma_start(dum_dram[:], dsc[:])

                if not ln_is_ones:
                    if SAFE_BCAST:
                        lnpad = sbT1.tile([128, FS], f32, tag="T1")
                        nc.vector.memset(lnpad[:], 0.0)
                        nc.sync.dma_start(lnpad[0:1, :], lnw_t.ap())
                        nc.gpsimd.partition_all_reduce(
                            Ln_bcast[:], lnpad[:], channels=128,
                            reduce_op=RO.add)
                    else:
                        lnr = sbC.tile([1, FS], f32)
                        nc.sync.dma_start(lnr[:], lnw_t.ap())
                        nc.gpsimd.partition_broadcast(Ln_bcast[:], lnr[:],
                                                      channels=128)

            # ================= Wg/Wu quantization stream =================
            # The last RESID stat pairs are still live in the 4-deep rings,
            # so the chain starts on them at c1 while the other 12 pairs
            # re-read; each re-read is emitted right after the chain step
            # whose ring slot it reuses, so the ring semaphores self-pace
            # the stream with prefetch distance 4 (no FIFO cycles).
            RESID = 4
            D_ORDER = list(range(DT - RESID, DT)) + list(range(DT - RESID))
            wq_g = [None] * DT
            wq_u = [None] * DT
            rrtiles = {}
            for j, dd in enumerate(D_ORDER):
                wts = stat_wts[dd] if j < RESID else rrtiles.pop(dd)
                for lst, scol in ((wq_g, 0), (wq_u, 1)):
                    wt = wts[scol]
                    t1 = sbT1.tile([128, FS], f32, tag="T1")
                    if scol == 0:
                        nc.scalar.activation(out=t1[:], in_=wt[:],
                                             func=AF.Copy,
                                             scale=s_w_col[:, scol:scol + 1],
                                             bias=MAGIC)
                    else:
                        nc.vector.tensor_scalar(
                            out=t1[:], in0=wt[:],
                            scalar1=s_w_col[:, scol:scol + 1],
                            scalar2=MAGIC, op0=OP.mult, op1=OP.add)
                    t2 = clip_step(t1)
                    wq = sbWQ.tile([128, FS], fp8, tag="wq")
                    nc.scalar.activation(out=wq[:], in_=t2[:],
                                         func=AF.Copy, bias=-MAGIC)
                    lst[dd] = wq
                if j + RESID < DT:
                    nd = D_ORDER[j + RESID]
                    pair = []
                    for jj, ten in enumerate((wgT_t, wuT_t)):
                        eng = nc.sync if jj == 0 else nc.scalar
                        wt = sbS8.tile([128, FS], f32, bufs=4,
                                       tag=("Sg" if jj == 0 else "Su"))
                        eng.dma_start(wt[:],
                                      ten.ap()[nd * 128:(nd + 1) * 128, :])
                        pair.append(wt)
                    rrtiles[nd] = pair

            # ============ Wd stats/quant helpers (run inside the loop) =====
            def emit_wd_dma(i):
                # half-slab [128 f, 1024 d]; halves alternate HWDGE queues
                eng = nc.sync if i % 2 == 0 else nc.scalar
                wt = sbS8.tile([128, FS], f32, bufs=4,
                               tag=("Sg" if i % 2 == 0 else "Su"),
                               name=f"wdr{i}")
                eng.dma_start(
                    wt[:], wdT_t.ap()[(i // 2) * 128:(i // 2 + 1) * 128,
                                      (i % 2) * FS:(i % 2 + 1) * FS])
                wd_tiles[i] = wt

            def emit_wd_stats(i0, i1):
                for i in range(i0, i1):
                    abs_stat(wd_tiles[i][:], wacc[:, 32 + i:33 + i],
                             on_scalar=(i % 2 == 1))

            def emit_c2():
                cold = sbCol.tile([128, 1], f32, tag="cd")
                nc.vector.tensor_reduce(out=cold[:], in_=wacc[:, 32:48],
                                        axis=AX.X, op=OP.add)
                c2gu = sbCol.tile([128, 1], f32, tag="c2gu")
                nc.gpsimd.partition_all_reduce(c2gu[:], cold[:], channels=128,
                                               reduce_op=RO.add)
                nc.vector.memset(c2_sb[:], 0.0)
                nc.vector.tensor_copy(c2_sb[:, 0:1], c2gu[0:1, 0:1])
                nc.sync.dma_start(c2_in[:], c2_sb[:])
                collective("AllReduce", OP.add, c2_in[:], c2_out[:])

            def emit_c2_scalars():
                c2g = sbC.tile([1, 8], f32)
                nc.gpsimd.dma_start(c2g[:], c2_out[:])
                mws = sbC.tile([1, 2], f32)
                nc.vector.tensor_scalar(out=mws[:, 0:1], in0=c2g[:, 0:1],
                                        scalar1=1.0 / (float(F) * D),
                                        scalar2=QEPS, op0=OP.mult, op1=OP.max)
                nc.vector.reciprocal(mws[:, 1:2], mws[:, 0:1])
                mwsc = sbC.tile([128, 2], f32)
                bcast_row(mwsc, mws, 2)
                nc.vector.tensor_copy(m_wd_col[:], mwsc[:, 0:1])
                nc.vector.tensor_copy(s_wd_col[:], mwsc[:, 1:2])

            def emit_wd_chain(i0, i1):
                # re-read + quantize Wd slabs, accumulate ternary column sums
                for i in range(i0, i1):
                    chs = []
                    for b in range(2):
                        eng = nc.sync if b == 0 else nc.scalar
                        wt = sbS8.tile([128, FS], f32, bufs=4,
                                       tag=("Sg" if b == 0 else "Su"),
                                       name=f"wdq{i}_{b}")
                        eng.dma_start(
                            wt[:], wdT_t.ap()[i * 128:(i + 1) * 128,
                                              b * FS:(b + 1) * FS])
                        t1 = sbT1.tile([128, FS], f32, tag="T1")
                        if b == 0:
                            nc.scalar.activation(
                                out=t1[:], in_=wt[:],
                                func=AF.Copy, scale=s_wd_col[:], bias=MAGIC)
                        else:
                            nc.vector.tensor_scalar(
                                out=t1[:], in0=wt[:],
                                scalar1=s_wd_col[:], scalar2=MAGIC,
                                op0=OP.mult, op1=OP.add)
                        t2 = clip_step(t1)
                        ch = sbCol.tile([128, 1], f32, tag=f"wdacc{b}")
                        wdq = sbT1.tile([128, FS], fp8, tag="wdq", bufs=2,
                                        name="wdq")
                        nc.scalar.activation(out=wdq[:], in_=t2[:],
                                             func=AF.Copy, bias=-MAGIC,
                                             accum_out=ch[:])
                        chs.append(ch)
                    nc.vector.tensor_tensor(out=Ssh_cols[:, i:i + 1],
                                            in0=chs[0][:], in1=chs[1][:],
                                            op=OP.add)

            def emit_sh():
                # S row -> broadcast down partitions
                nc.sync.dma_start(r128(srow_dram[:]), Ssh_cols[:])
                if SAFE_BCAST:
                    shpad = sbT1.tile([128, FS], f32, tag="T1")
                    nc.vector.memset(shpad[:], 0.0)
                    nc.sync.dma_start(shpad[0:1, :], srow_dram[:])
                    nc.gpsimd.partition_all_reduce(Sh_bcast[:], shpad[:],
                                                   channels=128,
                                                   reduce_op=RO.add)
                else:
                    srow = sbC.tile([1, FS], f32)
                    nc.sync.dma_start(srow[:], srow_dram[:])
                    nc.gpsimd.partition_broadcast(Sh_bcast[:], srow[:],
                                                  channels=128)

            # ================= post-stats (requant h, dot with S) =========
            # handles a LIST of contiguous segments with a single gathered
            # stat chain (the per-op overhead, especially DVE reciprocal,
            # dominates the tiny [128, SEG] math)
            def emit_post(segs):
                t0 = SEGB[segs[0]]
                SEG = SEGB[segs[-1] + 1] - t0
                ssq_g = sbCol.tile([128, SEG * NCORES], f32, tag="st_g1")
                am_g = sbCol.tile([128, SEG * NCORES], f32, tag="st_g2")
                off = 0
                for s in segs:
                    sw = SEGB[s + 1] - SEGB[s]
                    nc.sync.dma_start(
                        ssq_g[:, off * NCORES:(off + sw) * NCORES],
                        st_out[s][0:128, :])
                    nc.sync.dma_start(
                        am_g[:, off * NCORES:(off + sw) * NCORES],
                        st_out[s][128:256, :])
                    off += sw
                ssq12 = sbCol.tile([128, SEG], f32, tag="st_a")
                nc.vector.tensor_reduce(
                    out=ssq12[:],
                    in_=ssq_g[:].rearrange("p (i r) -> p i r", r=NCORES),
                    axis=AX.X, op=OP.add)
                am12 = sbCol.tile([128, SEG], f32, tag="st_b")
                nc.vector.tensor_reduce(
                    out=am12[:],
                    in_=am_g[:].rearrange("p (i r) -> p i r", r=NCORES),
                    axis=AX.X, op=OP.max)
                v = sbCol.tile([128, SEG], f32, tag="st_c")
                nc.vector.tensor_scalar(out=v[:], in0=ssq12[:],
                                        scalar1=1.0 / F, scalar2=EPS,
                                        op0=OP.mult, op1=OP.add)
                sv = sbCol.tile([128, SEG], f32, tag="st_d")
                nc.scalar.activation(out=sv[:], in_=v[:], func=AF.Sqrt)
                rs = sbCol.tile([128, SEG], f32, tag="st_e")
                nc.vector.reciprocal(rs[:], sv[:])
                rg = sbCol.tile([128, SEG], f32, tag="st_f")
                nc.vector.tensor_tensor(out=rg[:], in0=rs[:], in1=am12[:],
                                        op=OP.mult)
                y2 = sbCol.tile([128, SEG], f32, tag="st_g")
                nc.vector.tensor_scalar(out=y2[:], in0=rg[:], scalar1=QEPS,
                                        scalar2=None, op0=OP.max)
                invs2 = sbCol.tile([128, SEG], f32, tag="st_h")
                nc.vector.tensor_scalar(
                    out=invs2[:], in0=y2[:], scalar1=m_wd_col[:],
                    scalar2=1.0 / (127.0 * float(H) * D),
                    op0=OP.mult, op1=OP.mult)
                r2 = sbCol.tile([128, SEG], f32, tag="st_i")
                nc.vector.reciprocal(r2[:], y2[:])
                alpha = sbCol.tile([128, SEG], f32, tag="st_j")
                nc.vector.tensor_tensor(out=alpha[:], in0=r2[:], in1=rs[:],
                                        op=OP.mult)
                alpha2 = sbCol.tile([128, SEG], f32, tag="st_k")
                nc.vector.tensor_scalar(out=alpha2[:], in0=alpha[:],
                                        scalar1=127.0, scalar2=None,
                                        op0=OP.mult)
                for i in range(SEG):
                    t = t0 + i
                    w1 = sbT1.tile([128, FS], f32, tag="T1")
                    # requant magic-add on scalar (keeps vector free for the
                    # rowsum); the -MAGIC step stays on scalar too
                    nc.scalar.activation(out=w1[:], in_=ht_tiles[t][:],
                                         func=AF.Copy,
                                         scale=alpha2[:, i:i + 1], bias=MAGIC)
                    hq = sbT1.tile([128, FS], f32, tag="T1")
                    nc.scalar.activation(out=hq[:], in_=w1[:],
                                         func=AF.Identity, bias=negmagic[:])
                    qacc = sbCol.tile([128, 1], f32, tag="qacc")
                    mult_rowsum(hq[:], Sh_bcast[:], qacc, None)
                    nc.vector.tensor_scalar(out=Q_cols[:, t:t + 1],
                                            in0=qacc[:],
                                            scalar1=invs2[:, i:i + 1],
                                            scalar2=None, op0=OP.mult)

            # ================= main matmul loop =================
            with tc.tile_pool(name="psM", bufs=2, space="PSUM") as psM:
                nxt_xq = None
                for gi, (t0, gsz) in enumerate(GROUPS):
                    for tl in range(gsz):
                        t = t0 + tl
                        tc0 = tl * 128
                        gps = [psM.tile([128, NF], f32, tag=f"g{j}",
                                        name=f"gp{j}") for j in range(FH)]
                        ups = [psM.tile([128, NF], f32, tag=f"u{j}",
                                        name=f"up{j}") for j in range(FH)]
                        for di, d in enumerate(D_ORDER):
                            lhsT = cur_xq[d][:, tc0:tc0 + 128]
                            s0, s1 = (di == 0), (di == DT - 1)
                            for j in range(FH):
                                nc.tensor.matmul(gps[j][:], lhsT,
                                                 wq_g[d][:, j * NF:(j + 1) * NF],
                                                 start=s0, stop=s1)
                                nc.tensor.matmul(ups[j][:], lhsT,
                                                 wq_u[d][:, j * NF:(j + 1) * NF],
                                                 start=s0, stop=s1)
                        us = sbUG.tile([128, FS], fp16, tag="us")
                        gsl = sbUG.tile([128, FS], fp16, tag="gs")
                        for j in range(FH):
                            nc.scalar.activation(out=us[:, j * NF:(j + 1) * NF],
                                                 in_=ups[j][:], func=AF.Copy,
                                                 scale=DEQU[:, t:t + 1])
                            nc.scalar.activation(out=gsl[:, j * NF:(j + 1) * NF],
                                                 in_=gps[j][:], func=AF.Silu,
                                                 scale=DEQG[:, t:t + 1])
                        ht = sbH.tile([128, FS], fp16, tag="h")
                        ht_tiles[t] = ht
                        if ln_is_ones:
                            nc.vector.tensor_tensor(out=ht[:], in0=gsl[:],
                                                    in1=us[:], op=OP.mult)
                            hsq = sbUG.tile([128, FS], fp16, tag="hsq", bufs=1)
                            nc.scalar.activation(
                                out=hsq[:], in_=ht[:], func=AF.Square,
                                accum_out=ssq_cols[:, t:t + 1])
                            nc.vector.tensor_reduce(
                                out=am_cols[:, t:t + 1], in_=ht[:], axis=AX.X,
                                op=OP.max, apply_absolute_value=True)
                        else:
                            htf = sbT1.tile([128, FS], f32, tag="T1")
                            nc.vector.tensor_tensor(out=htf[:], in0=gsl[:],
                                                    in1=us[:], op=OP.mult)
                            hsq = sbUG.tile([128, FS], fp16, tag="hsq", bufs=1)
                            nc.scalar.activation(
                                out=hsq[:], in_=htf[:], func=AF.Square,
                                accum_out=ssq_cols[:, t:t + 1])
                            nc.vector.tensor_tensor(out=ht[:], in0=htf[:],
                                                    in1=Ln_bcast[:],
                                                    op=OP.mult)
                            nc.vector.tensor_reduce(
                                out=am_cols[:, t:t + 1], in_=ht[:], axis=AX.X,
                                op=OP.max, apply_absolute_value=True)
                        # segment boundary: slot stats, one AllReduce(max)
                        for s in range(NSEG):
                            if t == SEGB[s + 1] - 1:
                                a, b2 = SEGB[s], SEGB[s + 1]
                                seg = b2 - a
                                mrep = mask_bcast[:].unsqueeze(1) \
                                    .broadcast_to([128, seg, NCORES])
                                for ci, cols in enumerate((ssq_cols, am_cols)):
                                    slt = sbCol.tile([128, seg * NCORES], f32,
                                                     tag=f"slt{ci}",
                                                     name=f"slt{ci}")
                                    nc.vector.tensor_tensor(
                                        out=slt[:].rearrange(
                                            "p (i r) -> p i r", r=NCORES),
                                        in0=cols[:, a:b2].unsqueeze(2)
                                        .broadcast_to([128, seg, NCORES]),
                                        in1=mrep, op=OP.mult)
                                    nc.sync.dma_start(
                                        st_in[s][128 * ci:128 * (ci + 1), :],
                                        slt[:])
                                collective("AllReduce", OP.max,
                                           st_in[s][:], st_out[s][:])
                        # staggered Wd stat pass: the |.| stat for slab i
                        # runs two tiles after its DMA was emitted, so the
                        # 2-buf ring stays acyclic with loop prefetches
                        if 7 <= t <= 14:
                            emit_wd_stats(2 * (t - 7), 2 * (t - 7) + 2)
                        if 5 <= t <= 12:
                            emit_wd_dma(2 * (t - 5))
                            emit_wd_dma(2 * (t - 5) + 1)
                        if gi == 4 and tl == 0:
                            emit_c2_scalars()
                        if gi == 4 and tl >= 1:
                            emit_wd_chain(3 * (tl - 1), min(3 * tl, 8))
                        # prefetch next group's xq after the 2nd tile
                        if tl == min(1, gsz - 1) and gi + 1 < len(GROUPS):
                            nxt_xq = emit_xq(gi + 1, [None] * DT)
                    if gi == 3:
                        emit_c2()
                    if gi == 4:
                        emit_sh()
                    if POST_AT.get(gi):
                        emit_post(POST_AT[gi])
                    if gi + 1 < len(GROUPS):
                        cur_xq, nxt_xq = nxt_xq, None

            if POST_TAIL:
                emit_post(POST_TAIL)

            # ============ pooled partials + classifier ============
            with tc.tile_pool(name="psE", bufs=1, space="PSUM") as psE:
                pq = psE.tile([1, TT], f32, tag="pq")
                nc.tensor.matmul(pq[:], ones_col[:], Q_cols[:],
                                 start=True, stop=True)
                plrow = sbC.tile([1, TT], f32)
                nc.vector.tensor_copy(plrow[:], pq[:])
                nc.sync.dma_start(pl_in[:], plrow[:])
                collective("AllReduce", OP.add, pl_in[:], pl_out[:])

                pool3 = sbC.tile([C, B], f32)
                nc.sync.dma_start(
                    pool3[:], pl_out[:].rearrange("o (b c) -> (o c) b", c=C))
                clsW_sb = sbC.tile([C, NCLS], f32)
                nc.sync.dma_start(clsW_sb[:], clsWT_t.ap())
                clsb_sb = sbC.tile([1, NCLS], f32)
                nc.sync.dma_start(clsb_sb[:], clsb_t.ap())
                out_sb = sbC.tile([B, NCLS], f32)
                for j in range(0, NCLS, 512):
                    w = min(512, NCLS - j)
                    pcls = psE.tile([B, 512], f32, tag="pcls", bufs=2)
                    nc.tensor.matmul(pcls[:, 0:w], pool3[:],
                                     clsW_sb[:, j:j + w], start=True,
                                     stop=False)
                    nc.tensor.matmul(pcls[:, 0:w], ones1[:, 0:B],
                                     clsb_sb[:, j:j + w], start=False,
                                     stop=True)
                    nc.vector.tensor_copy(out_sb[:, j:j + w], pcls[:, 0:w])
                nc.sync.dma_start(out_t.ap(), out_sb[:])

    nc.compile()
    meta = dict(B=B, C=C, H=H, D=D, F=F, NCLS=NCLS, NCORES=NCORES,
                T=T, TS=TS, FS=FS)
    return nc, meta


def make_in_maps(x, Wg, Wu, Wd, ln_w, cls_W, cls_b, meta):
    """Host-side sharding: slices/transposes only, no arithmetic."""
    T, TS, FS = meta["T"], meta["TS"], meta["FS"]
    D = meta["D"]
    NCLS = meta["NCLS"]
    NCORES = meta["NCORES"]
    xf = np.ascontiguousarray(np.asarray(x, np.float32).reshape(T, D))
    xT = np.ascontiguousarray(xf.T)
    clsWT = np.ascontiguousarray(np.asarray(cls_W, np.float32).T)
    clsb2 = np.ascontiguousarray(np.asarray(cls_b, np.float32).reshape(1, NCLS))
    maps = []
    for k in range(NCORES):
        f0 = k * FS
        cmask = np.zeros((1, NCORES), np.float32)
        cmask[0, k] = 1.0
        maps.append({
            "xs": np.ascontiguousarray(xf[k * TS:(k + 1) * TS]),
            "xT": xT,
            "wgT": np.ascontiguousarray(np.asarray(Wg, np.float32)[f0:f0 + FS, :].T),
            "wuT": np.ascontiguousarray(np.asarray(Wu, np.float32)[f0:f0 + FS, :].T),
            "wdT": np.ascontiguousarray(np.asarray(Wd, np.float32)[:, f0:f0 + FS].T),
            "lnw": np.ascontiguousarray(np.asarray(ln_w, np.float32)[f0:f0 + FS].reshape(1, FS)),
            "clsWT": clsWT,
            "clsb": clsb2,
            "cmask": cmask,
        })
    return maps


_CACHE = {}


def kernel(x, Wg, Wu, Wd, ln_w, cls_W, cls_b):
    """Takes FULL inputs, runs the 8-core SPMD Bass kernel, returns [B, NCLS]."""
    from concourse import bass_utils

    x = np.asarray(x, np.float32)
    B, C, H, D = x.shape
    F = int(np.asarray(Wg).shape[0])
    NCLS = int(np.asarray(cls_W).shape[0])
    ln_ones = bool(np.all(np.asarray(ln_w) == 1.0))
    key = (B, C, H, D, F, NCLS, ln_ones)
    if key not in _CACHE:
        _CACHE[key] = build(B=B, C=C, H=H, D=D, F=F, NCLS=NCLS, NCORES=8,
                            ln_is_ones=ln_ones)
    nc, meta = _CACHE[key]
    in_maps = make_in_maps(x, Wg, Wu, Wd, ln_w, cls_W, cls_b, meta)
    res = bass_utils.run_bass_kernel_spmd(nc, in_maps, core_ids=list(range(8)))
    return np.asarray(res.results[0]["out"], np.float32)


# revision 58
# speedup vs baseline: 1.0159x; 1.0159x over previous
"""BitNet SwiGLU MLP kernel for Trainium2, tensor-parallel over 8 NeuronCores.

Sharding (Megatron-style TP over the intermediate dim F):
- Each core holds a 1/8 column-shard of Wg/Wu (fed transposed: [D, FS]) and
  the matching shard of Wd (fed as Wd[:, shard].T = [FS, D]). x is replicated,
  fed both natural-sliced (per-token quant stats, sharded over tokens) and
  fully transposed [D, T] (matmul operand layout).
- bit_linear runs as an exact integer matmul: quantized activations are ints
  in [-128,127] (bf16 lhsT) and ternary weights in {-1,0,1} (fp8e4 rhs, both
  exact, accumulated exactly in fp32 PSUM), dequantized on the output by
  per-token / global scales. clip(round(.)) for activations is exact RNE via
  the fp32 magic-number trick (the clip never binds since |x*scale| <= 127).
- Down-proj + mean-pool is collapsed algebraically:
  mean_{h,d}(hq @ Wdq.T) = 1/(H*D) * sum_f hq[t,f] * S[f],  S = colsum(Wdq)
  so only a per-token weighted row-reduction against S remains.
- h is kept resident in SBUF as fp16 between the main loop and the
  per-token requantization pass; no DRAM roundtrip.
- Head is latency-optimized: weight |.| stats stream dual-engine
  (vector+scalar) as DMA lands, cross-partition sums go through gpsimd (the
  PE never sits on the c1 critical path), the quant-pass re-read is
  prefetched during the c1 AllReduce wait, and the Wd stat/quant work is
  deferred into the early loop groups where engines have slack.
- Per-segment RMS/absmax stats cross cores via ONE AllReduce(max) with
  per-core slots (mask built from a one-hot input); pooled partials are
  AllReduced at the end; every core runs the tiny classifier.
"""
import numpy as np

MAGIC = 12582912.0  # 1.5 * 2^23, fp32 RNE magic
EPS = 1e-6
QEPS = 1e-5


def build(B=8, C=3, H=128, D=2048, F=8192, NCLS=1000, NCORES=8,
          ln_is_ones=True, mock_collectives=False,
          NDUM1=480, NDUM2=55, H_BUFS=20, XQ_BUFS=20, RQ_BUFS=2,
          SAFE_CLIP=False, SAFE_BCAST=True, SAFE_TTR=True):
    """Build + compile the SPMD Bass program. Returns (nc, meta)."""
    import concourse.bacc as bacc
    import concourse.tile as tile
    from concourse import mybir
    from concourse import bass_isa

    f32 = mybir.dt.float32
    bf16 = mybir.dt.bfloat16
    fp16 = mybir.dt.float16
    fp8 = mybir.dt.float8e4
    AX = mybir.AxisListType
    OP = mybir.AluOpType
    AF = mybir.ActivationFunctionType
    RO = bass_isa.ReduceOp
    RG = [list(range(NCORES))]

    assert H == 128
    T = B * C * H
    TT = T // 128               # token tiles (== B*C) = 24
    TS = T // NCORES            # tokens per core for x stats
    TST = TS // 128
    FS = F // NCORES            # f-shard width = 1024
    DT = D // 128               # contraction tiles = 16
    NF = 512
    FH = FS // NF               # = 2
    WB = 2                      # d-tiles per weight/x DMA slab

    # token-tile groups (last ones smaller to tighten the tail)
    GROUPS = [(0, 4), (4, 4), (8, 4), (12, 4), (16, 4), (20, 2), (22, 2)]
    assert sum(g[1] for g in GROUPS) == TT
    SEGB = [0, 4, 8, 12, 16, 20, 23, 24]
    NSEG = len(SEGB) - 1
    def group_of(t):
        for gi, (t0, gsz) in enumerate(GROUPS):
            if t0 <= t < t0 + gsz:
                return gi
        raise AssertionError
    # post-stats for segment s run AFTER group gpost's tiles (so an engine
    # FIFO wait on the segment's collective can never block later tiles'
    # compute that feeds later collectives). Early segments get extra slack
    # because Sh_bcast (built from the in-loop Wd quant) lands around g3.
    # Sh_bcast (from the in-loop Wd quant chain) lands at the end of group
    # 4, so the early segments all post right after it; h tiles are fully
    # SBUF-resident so late posts never throttle the loop.
    POST_AT = {4: [0, 1, 2], 5: [3], 6: [4, 5]}
    POST_TAIL = [6]

    nc = bacc.Bacc("TRN2", target_bir_lowering=False, debug=False,
                   num_devices=1 if mock_collectives else NCORES)

    def collective(kind, op, in_ap, out_ap):
        if NCORES == 1 or mock_collectives:
            n = out_ap.size() // in_ap.size()
            flat = out_ap.rearrange("a b -> (a b)")
            for r in range(n):
                nc.sync.dma_start(
                    flat[r * in_ap.size():(r + 1) * in_ap.size()], in_ap)
        else:
            nc.gpsimd.collective_compute(kind, op, replica_groups=RG,
                                         ins=[in_ap.opt()], outs=[out_ap.opt()])

    xs_t = nc.dram_tensor("xs", [TS, D], f32, kind="ExternalInput")
    xT_t = nc.dram_tensor("xT", [D, T], f32, kind="ExternalInput")
    wgT_t = nc.dram_tensor("wgT", [D, FS], f32, kind="ExternalInput")
    wuT_t = nc.dram_tensor("wuT", [D, FS], f32, kind="ExternalInput")
    wdT_t = nc.dram_tensor("wdT", [FS, D], f32, kind="ExternalInput")
    lnw_t = nc.dram_tensor("lnw", [1, FS], f32, kind="ExternalInput")
    clsWT_t = nc.dram_tensor("clsWT", [C, NCLS], f32, kind="ExternalInput")
    clsb_t = nc.dram_tensor("clsb", [1, NCLS], f32, kind="ExternalInput")
    cmask_t = nc.dram_tensor("cmask", [1, NCORES], f32, kind="ExternalInput")
    out_t = nc.dram_tensor("out", [B, NCLS], f32, kind="ExternalOutput")

    def r128(ap):
        # [1, n*128] dram view -> [128, n] (partition = fast axis)
        return ap.rearrange("o (i p) -> (o p) i", p=128)

    with tile.TileContext(nc) as tc:
        import contextlib
        with contextlib.ExitStack() as st:
            dram = st.enter_context(tc.tile_pool(name="dram", bufs=1, space="DRAM"))
            sbC = st.enter_context(tc.tile_pool(name="sbC", bufs=1))
            sbS8 = st.enter_context(tc.tile_pool(name="sbS8", bufs=2))
            sbT1 = st.enter_context(tc.tile_pool(name="sbT1", bufs=2))
            sbS2 = st.enter_context(tc.tile_pool(name="sbS2", bufs=3))
            sbUG = st.enter_context(tc.tile_pool(name="sbUG", bufs=2))
            sbCol = st.enter_context(tc.tile_pool(name="sbCol", bufs=4))
            sbH = st.enter_context(tc.tile_pool(name="sbH", bufs=H_BUFS))
            sbXQ = st.enter_context(tc.tile_pool(name="sbXQ", bufs=XQ_BUFS))
            sbWQ = st.enter_context(tc.tile_pool(name="sbWQ", bufs=2 * DT))

            sc_in = dram.tile([1, TS], f32)
            sc_out = dram.tile([1, T], f32)
            c1_in = dram.tile([1, 8], f32)
            c1_out = dram.tile([1, 8], f32)
            c2_in = dram.tile([1, 8], f32)
            c2_out = dram.tile([1, 8], f32)
            srow_dram = dram.tile([1, FS], f32)
            srow2_dram = dram.tile([1, T], f32)
            dum_dram = dram.tile([1, 8], f32)
            # slotted stats exchange: one AllReduce(max) per segment over
            # [256, SEG*8]; core k's ssq/am partials sit in slot k of the
            # innermost axis (all other slots zero, and partials are >= 0,
            # so max == gather). Local free-axis reduce then combines slots.
            st_in = [dram.tile([256, (SEGB[s + 1] - SEGB[s]) * NCORES], f32,
                               name=f"st_in{s}") for s in range(NSEG)]
            st_out = [dram.tile([256, (SEGB[s + 1] - SEGB[s]) * NCORES], f32,
                                name=f"st_out{s}", addr_space="Shared")
                      for s in range(NSEG)]
            pl_in = dram.tile([1, TT], f32)
            pl_out = dram.tile([1, TT], f32, addr_space="Shared")

            ones1 = sbC.tile([1, 128], f32)
            nc.vector.memset(ones1[:], 1.0)
            ones_col = sbC.tile([128, 1], f32)
            nc.vector.memset(ones_col[:], 1.0)
            negmagic = sbC.tile([128, 1], f32)
            nc.vector.memset(negmagic[:], -MAGIC)
            zeros_bf = sbC.tile([128, NF], bf16)
            nc.vector.memset(zeros_bf[:], 0.0)

            wacc = sbC.tile([128, 48], f32)
            mask_bcast = sbC.tile([128, NCORES], f32)
            sc_cols = sbC.tile([128, TST], f32)
            c1_sb = sbC.tile([1, 8], f32)
            c2_sb = sbC.tile([1, 8], f32)
            m_w_col = sbC.tile([128, 2], f32)
            s_w_col = sbC.tile([128, 2], f32)
            m_wd_col = sbC.tile([128, 1], f32)
            s_wd_col = sbC.tile([128, 1], f32)
            S_bcast = sbC.tile([128, T], f32)
            Sh_bcast = sbC.tile([128, FS], f32)
            DEQG = sbC.tile([128, TT], f32)
            DEQU = sbC.tile([128, TT], f32)
            ssq_cols = sbC.tile([128, TT], f32)
            am_cols = sbC.tile([128, TT], f32)
            Q_cols = sbC.tile([128, TT], f32)
            Ssh_cols = sbC.tile([128, FS // 128], f32)
            if not ln_is_ones:
                Ln_bcast = sbC.tile([128, FS], f32)

            ht_tiles = [None] * TT

            def bcast_row(out_cols, in_row, n):
                # out_cols [128, n] <- broadcast of in_row [1, n]
                if SAFE_BCAST:
                    pad = sbCol.tile([128, max(n, 1)], f32, tag="bc",
                                     bufs=2, name="bcpad")
                    nc.vector.memset(pad[:, 0:n], 0.0)
                    nc.vector.tensor_copy(pad[0:1, 0:n], in_row[0:1, 0:n])
                    nc.gpsimd.partition_all_reduce(out_cols[:, 0:n],
                                                   pad[:, 0:n], channels=128,
                                                   reduce_op=RO.add)
                else:
                    nc.gpsimd.partition_broadcast(out_cols[:, 0:n],
                                                  in_row[0:1, 0:n],
                                                  channels=128)

            def clip_step(t1):
                # clamp t1 (rounded magic form) to [MAGIC-1, MAGIC+1]
                if SAFE_CLIP:
                    t2 = sbT1.tile([128, FS], f32, tag="T2", bufs=2)
                    nc.vector.tensor_scalar(out=t2[:], in0=t1[:],
                                            scalar1=MAGIC - 1.0,
                                            scalar2=MAGIC + 1.0,
                                            op0=OP.max, op1=OP.min)
                    return t2
                nc.vector.tensor_scalar(out=t1[:], in0=t1[:],
                                        scalar1=MAGIC - 1.0,
                                        scalar2=MAGIC + 1.0,
                                        op0=OP.max, op1=OP.min)
                return t1

            def mult_rowsum(in0, in1, accum, init):
                # accum [128,1] = init + sum_cols(in0 * in1)
                if SAFE_TTR:
                    jk = sbUG.tile([128, FS], bf16, tag="jkf", bufs=2,
                                   name="jkf")
                    nc.vector.tensor_tensor(out=jk[:], in0=in0, in1=in1,
                                            op=OP.mult)
                    if init is None:
                        nc.vector.tensor_reduce(out=accum[:], in_=jk[:],
                                                axis=AX.X, op=OP.add)
                    else:
                        part = sbCol.tile([128, 1], f32, tag="mrs",
                                          name="mrs")
                        nc.vector.tensor_reduce(out=part[:], in_=jk[:],
                                                axis=AX.X, op=OP.add)
                        nc.vector.tensor_tensor(out=accum[:], in0=part[:],
                                                in1=init[:], op=OP.add)
                else:
                    jk = sbUG.tile([128, FS], bf16, tag="jk", bufs=2,
                                   name="jk")
                    nc.vector.tensor_tensor_reduce(
                        out=jk[:], in0=in0, in1=in1, scale=1.0,
                        scalar=(0.0 if init is None else init[:]),
                        op0=OP.mult, op1=OP.add, accum_out=accum[:])

            def abs_stat(wt, wcol, on_scalar):
                # wcol [128,1] = sum_cols |wt|, on either engine
                if on_scalar:
                    # the elementwise output is never read (fp8 scratch);
                    # only the fp32 row-accumulator matters
                    wsc = sbT1.tile([128, wt.free_size()], fp8, tag="wsc",
                                    bufs=2, name="wsc")
                    nc.scalar.activation(out=wsc[:], in_=wt, func=AF.Abs,
                                         accum_out=wcol)
                else:
                    nc.vector.tensor_reduce(out=wcol, in_=wt, axis=AX.X,
                                            op=OP.add,
                                            apply_absolute_value=True)

            # emit_xq defined up-front: the head emits group 0 itself
            def emit_xq(gi, xq_slabs):
                t0, gsz = GROUPS[gi]
                gw = gsz * 128
                tc0 = t0 * 128
                for d0 in range(0, DT, WB):
                    xsl = sbXQ.tile([128, WB * 512], f32, tag="xsl", bufs=2)
                    nc.sync.dma_start(
                        xsl[:, 0:WB * gw],
                        xT_t.ap()[d0 * 128:(d0 + WB) * 128, tc0:tc0 + gw]
                        .rearrange("(b p) c -> p b c", p=128))
                    for b in range(WB):
                        xp = sbXQ.tile([128, 512], f32, tag="xp", bufs=2)
                        nc.vector.tensor_tensor(
                            out=xp[:, 0:gw], in0=xsl[:, b * gw:(b + 1) * gw],
                            in1=S_bcast[:, tc0:tc0 + gw], op=OP.mult)
                        xq = sbXQ.tile([128, 512], bf16, tag="xq")
                        nc.vector.tensor_scalar(out=xq[:, 0:gw],
                                                in0=xp[:, 0:gw],
                                                scalar1=MAGIC, scalar2=MAGIC,
                                                op0=OP.add, op1=OP.subtract)
                        xq_slabs[d0 + b] = xq
                return xq_slabs

            # ================= Head =================
            wd_tiles = [None] * (2 * (FS // 128))
            with tc.tile_pool(name="psH", bufs=1, space="PSUM") as psH:
                # dummy matmuls keep the PE p-state up until real MMs arrive
                if NDUM1 > 0:
                    dum = psH.tile([128, NF], f32, tag="dum")
                    for k in range(NDUM1):
                        nc.tensor.matmul(dum[:], zeros_bf[:, 0:128],
                                         zeros_bf[:], start=(k == 0),
                                         stop=(k == NDUM1 - 1))

                # x per-token clipped absmax -> sc_in -> AllGather (first
                # collective; also warms the CC stream behind the barrier).
                # Halved DMAs ride both hardware DGE queues concurrently.
                for i in range(TST):
                    hs = []
                    for h2 in range(2):
                        xt = sbS8.tile([128, FS], f32, bufs=4,
                                       tag=("Sg" if h2 == 0 else "Su"))
                        nc.gpsimd.dma_start(
                            xt[:], xs_t.ap()[i * 128:(i + 1) * 128,
                                             h2 * FS:(h2 + 1) * FS])
                        hs.append(xt)
                    am = sbCol.tile([128, 2], f32, tag="am")
                    for h2 in range(2):
                        nc.vector.tensor_reduce(out=am[:, h2:h2 + 1],
                                                in_=hs[h2][:], axis=AX.X,
                                                op=OP.max,
                                                apply_absolute_value=True)
                    am1 = sbCol.tile([128, 1], f32, tag="am1")
                    nc.vector.tensor_reduce(out=am1[:], in_=am[:], axis=AX.X,
                                            op=OP.max)
                    nc.vector.tensor_scalar(out=sc_cols[:, i:i + 1], in0=am1[:],
                                            scalar1=QEPS, scalar2=None,
                                            op0=OP.max)
                nc.sync.dma_start(r128(sc_in[:]), sc_cols[:])
                collective("AllGather", OP.bypass, sc_in[:], sc_out[:])

                # Wg/Wu |.| sums. Two hardware DGE queues run concurrently:
                # g slabs load via SP (reduce on vector), u slabs via the
                # Activation HWDGE (reduce on scalar). Stats lag their slab
                # by one iteration so neither engine FIFO blocks the other
                # queue's triggers.
                stat_q = []
                stat_wts = {}
                for d in range(DT):
                    wts = []
                    for j, ten in enumerate((wgT_t, wuT_t)):
                        eng = nc.sync if j == 0 else nc.scalar
                        wt = sbS8.tile([128, FS], f32, bufs=4,
                                       tag=("Sg" if j == 0 else "Su"))
                        eng.dma_start(wt[:],
                                      ten.ap()[d * 128:(d + 1) * 128, :])
                        wts.append(wt)
                    stat_wts[d] = wts
                    stat_q.append((d, wts))
                    if len(stat_q) > 1:
                        pi, pw = stat_q.pop(0)
                        for j in range(2):
                            abs_stat(pw[j][:], wacc[:, j * 16 + pi:j * 16 + pi + 1],
                                     on_scalar=False)
                for pi, pw in stat_q:
                    for j in range(2):
                        abs_stat(pw[j][:], wacc[:, j * 16 + pi:j * 16 + pi + 1],
                                 on_scalar=False)
                # cross-partition totals via gpsimd (PE stays off this path)
                colgu = sbCol.tile([128, 2], f32, tag="cgu")
                nc.vector.tensor_reduce(out=colgu[:, 0:1], in_=wacc[:, 0:16],
                                        axis=AX.X, op=OP.add)
                nc.vector.tensor_reduce(out=colgu[:, 1:2], in_=wacc[:, 16:32],
                                        axis=AX.X, op=OP.add)
                c1gu = sbCol.tile([128, 2], f32, tag="c1gu")
                nc.gpsimd.partition_all_reduce(c1gu[:], colgu[:], channels=128,
                                               reduce_op=RO.add)
                nc.vector.memset(c1_sb[:], 0.0)
                nc.vector.tensor_copy(c1_sb[:, 0:2], c1gu[0:1, 0:2])
                nc.sync.dma_start(c1_in[:], c1_sb[:])
                collective("AllReduce", OP.add, c1_in[:], c1_out[:])

                # S_bcast = 127 / absmax: the reciprocal runs on the
                # compact [128, TT] gathered form (DVE reciprocal is slow),
                # then a DRAM bounce re-rows it and the PE broadcasts it
                # down partitions. Emitted before emit_xq (vector FIFO
                # ordering). Numerically identical to reciprocal-then-scale
                # on the broadcast form.
                yraw = sbC.tile([128, TT], f32)
                nc.gpsimd.dma_start(yraw[:], r128(sc_out[:]))
                r24 = sbC.tile([128, TT], f32)
                nc.vector.reciprocal(r24[:], yraw[:])
                s24 = sbC.tile([128, TT], f32)
                nc.vector.tensor_scalar(out=s24[:], in0=r24[:],
                                        scalar1=127.0, scalar2=None,
                                        op0=OP.mult)
                nc.sync.dma_start(r128(srow2_dram[:]), s24[:])
                for j in range(0, T, 512):
                    scr = sbS2.tile([1, 512], f32, tag="scr")
                    nc.gpsimd.dma_start(scr[:], srow2_dram[0:1, j:j + 512])
                    pb = psH.tile([128, 512], f32, tag="pb", bufs=2)
                    nc.tensor.matmul(pb[:], ones1[:], scr[:],
                                     start=True, stop=True)
                    nc.scalar.activation(out=S_bcast[:, j:j + 512],
                                         in_=pb[:], func=AF.Copy)

                # group-0 xq: xT DMAs queue right behind the stat reads
                cur_xq = emit_xq(0, [None] * DT)

                # ---- derived scalars from c1 (g,u) ----
                # (these DMAs are emitted BEFORE the rq stream below so no
                # rq slab -- whose buffer release depends on them -- can sit
                # ahead of them in a DMA queue: that would deadlock)
                c1g = sbC.tile([1, 8], f32)
                nc.gpsimd.dma_start(c1g[:], c1_out[:])
                m_w = sbC.tile([1, 2], f32)
                nc.vector.tensor_scalar(out=m_w[:], in0=c1g[:, 0:2],
                                        scalar1=1.0 / (float(F) * D),
                                        scalar2=QEPS, op0=OP.mult, op1=OP.max)
                s_w = sbC.tile([1, 2], f32)
                nc.vector.reciprocal(s_w[:], m_w[:])
                bcast_row(m_w_col, m_w, 2)
                bcast_row(s_w_col, s_w, 2)

                nc.vector.tensor_scalar(out=DEQG[:], in0=yraw[:],
                                        scalar1=m_w_col[:, 0:1],
                                        scalar2=1.0 / 127.0,
                                        op0=OP.mult, op1=OP.mult)
                nc.vector.tensor_scalar(out=DEQU[:], in0=yraw[:],
                                        scalar1=m_w_col[:, 1:2],
                                        scalar2=1.0 / 127.0,
                                        op0=OP.mult, op1=OP.mult)

                # per-core one-hot slot mask (c1-wait window)
                cm_row = sbC.tile([1, NCORES], f32)
                nc.sync.dma_start(cm_row[:], cmask_t.ap())
                bcast_row(mask_bcast, cm_row, NCORES)

                # second dummy batch bridges the gap to the first real MMs
                if NDUM1 > 0 and NDUM2 > 0:
                    dum2 = psH.tile([128, NF], f32, tag="dum2")
                    for k in range(NDUM2):
                        nc.tensor.matmul(dum2[:], zeros_bf[:, 0:128],
                                         zeros_bf[:], start=(k == 0),
                                         stop=(k == NDUM2 - 1))
                    dsc = sbC.tile([1, 8], f32)
                    nc.vector.memset(dsc[:], 0.0)
                    nc.vector.tensor_copy(dsc[:, 0:1], dum[0:1, 0:1])
                    nc.vector.tensor_copy(dsc[:, 1:2], dum2[0:1, 0:1])
                    nc.sync.dma_start(dum_dram[:], dsc[:])

                if not ln_is_ones:
                    if SAFE_BCAST:
                        lnpad = sbT1.tile([128, FS], f32, tag="T1")
                        nc.vector.memset(lnpad[:], 0.0)
                        nc.sync.dma_start(lnpad[0:1, :], lnw_t.ap())
                        nc.gpsimd.partition_all_reduce(
                            Ln_bcast[:], lnpad[:], channels=128,
                            reduce_op=RO.add)
                    else:
                        lnr = sbC.tile([1, FS], f32)
                        nc.sync.dma_start(lnr[:], lnw_t.ap())
                        nc.gpsimd.partition_broadcast(Ln_bcast[:], lnr[:],
                                                      channels=128)

            # ================= Wg/Wu quantization stream =================
            # The last RESID stat pairs are still live in the 4-deep rings,
            # so the chain starts on them at c1 while the other 12 pairs
            # re-read; each re-read is emitted right after the chain step
            # whose ring slot it reuses, so the ring semaphores self-pace
            # the stream with prefetch distance 4 (no FIFO cycles).
            RESID = 4
            D_ORDER = list(range(DT - RESID, DT)) + list(range(DT - RESID))
            wq_g = [None] * DT
            wq_u = [None] * DT
            rrtiles = {}
            for j, dd in enumerate(D_ORDER):
                wts = stat_wts[dd] if j < RESID else rrtiles.pop(dd)
                for lst, scol in ((wq_g, 0), (wq_u, 1)):
                    wt = wts[scol]
                    t1 = sbT1.tile([128, FS], f32, tag="T1")
                    if scol == 0:
                        nc.scalar.activation(out=t1[:], in_=wt[:],
                                             func=AF.Copy,
                                             scale=s_w_col[:, scol:scol + 1],
                                             bias=MAGIC)
                    else:
                        nc.vector.tensor_scalar(
                            out=t1[:], in0=wt[:],
                            scalar1=s_w_col[:, scol:scol + 1],
                            scalar2=MAGIC, op0=OP.mult, op1=OP.add)
                    t2 = clip_step(t1)
                    wq = sbWQ.tile([128, FS], fp8, tag="wq")
                    nc.scalar.activation(out=wq[:], in_=t2[:],
                                         func=AF.Copy, bias=-MAGIC)
                    lst[dd] = wq
                if j + RESID < DT:
                    nd = D_ORDER[j + RESID]
                    pair = []
                    for jj, ten in enumerate((wgT_t, wuT_t)):
                        eng = nc.sync if jj == 0 else nc.scalar
                        wt = sbS8.tile([128, FS], f32, bufs=4,
                                       tag=("Sg" if jj == 0 else "Su"))
                        eng.dma_start(wt[:],
                                      ten.ap()[nd * 128:(nd + 1) * 128, :])
                        pair.append(wt)
                    rrtiles[nd] = pair

            # ============ Wd stats/quant helpers (run inside the loop) =====
            def emit_wd_dma(i):
                # half-slab [128 f, 1024 d]; rides the gpsimd DGE ring so it
                # never contends with the quant re-read on the HWDGE queues
                wt = sbS8.tile([128, FS], f32, bufs=4,
                               tag=("Sg" if i % 2 == 0 else "Su"),
                               name=f"wdr{i}")
                nc.gpsimd.dma_start(
                    wt[:], wdT_t.ap()[(i // 2) * 128:(i // 2 + 1) * 128,
                                      (i % 2) * FS:(i % 2 + 1) * FS])
                wd_tiles[i] = wt

            def emit_wd_stats(i0, i1):
                for i in range(i0, i1):
                    abs_stat(wd_tiles[i][:], wacc[:, 32 + i:33 + i],
                             on_scalar=(i % 2 == 1))

            def emit_c2():
                cold = sbCol.tile([128, 1], f32, tag="cd")
                nc.vector.tensor_reduce(out=cold[:], in_=wacc[:, 32:48],
                                        axis=AX.X, op=OP.add)
                c2gu = sbCol.tile([128, 1], f32, tag="c2gu")
                nc.gpsimd.partition_all_reduce(c2gu[:], cold[:], channels=128,
                                               reduce_op=RO.add)
                nc.vector.memset(c2_sb[:], 0.0)
                nc.vector.tensor_copy(c2_sb[:, 0:1], c2gu[0:1, 0:1])
                nc.sync.dma_start(c2_in[:], c2_sb[:])
                collective("AllReduce", OP.add, c2_in[:], c2_out[:])

            def emit_c2_scalars():
                c2g = sbC.tile([1, 8], f32)
                nc.gpsimd.dma_start(c2g[:], c2_out[:])
                mws = sbC.tile([1, 2], f32)
                nc.vector.tensor_scalar(out=mws[:, 0:1], in0=c2g[:, 0:1],
                                        scalar1=1.0 / (float(F) * D),
                                        scalar2=QEPS, op0=OP.mult, op1=OP.max)
                nc.vector.reciprocal(mws[:, 1:2], mws[:, 0:1])
                mwsc = sbC.tile([128, 2], f32)
                bcast_row(mwsc, mws, 2)
                nc.vector.tensor_copy(m_wd_col[:], mwsc[:, 0:1])
                nc.vector.tensor_copy(s_wd_col[:], mwsc[:, 1:2])

            def emit_wd_chain(i0, i1):
                # re-read + quantize Wd slabs, accumulate ternary column sums
                for i in range(i0, i1):
                    chs = []
                    for b in range(2):
                        eng = nc.sync if b == 0 else nc.scalar
                        wt = sbS8.tile([128, FS], f32, bufs=4,
                                       tag=("Sg" if b == 0 else "Su"),
                                       name=f"wdq{i}_{b}")
                        eng.dma_start(
                            wt[:], wdT_t.ap()[i * 128:(i + 1) * 128,
                                              b * FS:(b + 1) * FS])
                        t1 = sbT1.tile([128, FS], f32, tag="T1")
                        if b == 0:
                            nc.scalar.activation(
                                out=t1[:], in_=wt[:],
                                func=AF.Copy, scale=s_wd_col[:], bias=MAGIC)
                        else:
                            nc.vector.tensor_scalar(
                                out=t1[:], in0=wt[:],
                                scalar1=s_wd_col[:], scalar2=MAGIC,
                                op0=OP.mult, op1=OP.add)
                        t2 = clip_step(t1)
                        ch = sbCol.tile([128, 1], f32, tag=f"wdacc{b}")
                        wdq = sbT1.tile([128, FS], fp8, tag="wdq", bufs=2,
                                        name="wdq")
                        nc.scalar.activation(out=wdq[:], in_=t2[:],
                                             func=AF.Copy, bias=-MAGIC,
                                             accum_out=ch[:])
                        chs.append(ch)
                    nc.vector.tensor_tensor(out=Ssh_cols[:, i:i + 1],
                                            in0=chs[0][:], in1=chs[1][:],
                                            op=OP.add)

            def emit_sh():
                # S row -> broadcast down partitions
                nc.sync.dma_start(r128(srow_dram[:]), Ssh_cols[:])
                if SAFE_BCAST:
                    shpad = sbT1.tile([128, FS], f32, tag="T1")
                    nc.vector.memset(shpad[:], 0.0)
                    nc.sync.dma_start(shpad[0:1, :], srow_dram[:])
                    nc.gpsimd.partition_all_reduce(Sh_bcast[:], shpad[:],
                                                   channels=128,
                                                   reduce_op=RO.add)
                else:
                    srow = sbC.tile([1, FS], f32)
                    nc.sync.dma_start(srow[:], srow_dram[:])
                    nc.gpsimd.partition_broadcast(Sh_bcast[:], srow[:],
                                                  channels=128)

            # ================= post-stats (requant h, dot with S) =========
            # handles a LIST of contiguous segments with a single gathered
            # stat chain (the per-op overhead, especially DVE reciprocal,
            # dominates the tiny [128, SEG] math)
            def emit_post(segs):
                t0 = SEGB[segs[0]]
                SEG = SEGB[segs[-1] + 1] - t0
                ssq_g = sbCol.tile([128, SEG * NCORES], f32, tag="st_g1")
                am_g = sbCol.tile([128, SEG * NCORES], f32, tag="st_g2")
                off = 0
                for s in segs:
                    sw = SEGB[s + 1] - SEGB[s]
                    nc.sync.dma_start(
                        ssq_g[:, off * NCORES:(off + sw) * NCORES],
                        st_out[s][0:128, :])
                    nc.sync.dma_start(
                        am_g[:, off * NCORES:(off + sw) * NCORES],
                        st_out[s][128:256, :])
                    off += sw
                ssq12 = sbCol.tile([128, SEG], f32, tag="st_a")
                nc.vector.tensor_reduce(
                    out=ssq12[:],
                    in_=ssq_g[:].rearrange("p (i r) -> p i r", r=NCORES),
                    axis=AX.X, op=OP.add)
                am12 = sbCol.tile([128, SEG], f32, tag="st_b")
                nc.vector.tensor_reduce(
                    out=am12[:],
                    in_=am_g[:].rearrange("p (i r) -> p i r", r=NCORES),
                    axis=AX.X, op=OP.max)
                v = sbCol.tile([128, SEG], f32, tag="st_c")
                nc.vector.tensor_scalar(out=v[:], in0=ssq12[:],
                                        scalar1=1.0 / F, scalar2=EPS,
                                        op0=OP.mult, op1=OP.add)
                sv = sbCol.tile([128, SEG], f32, tag="st_d")
                nc.scalar.activation(out=sv[:], in_=v[:], func=AF.Sqrt)
                rs = sbCol.tile([128, SEG], f32, tag="st_e")
                nc.vector.reciprocal(rs[:], sv[:])
                rg = sbCol.tile([128, SEG], f32, tag="st_f")
                nc.vector.tensor_tensor(out=rg[:], in0=rs[:], in1=am12[:],
                                        op=OP.mult)
                y2 = sbCol.tile([128, SEG], f32, tag="st_g")
                nc.vector.tensor_scalar(out=y2[:], in0=rg[:], scalar1=QEPS,
                                        scalar2=None, op0=OP.max)
                invs2 = sbCol.tile([128, SEG], f32, tag="st_h")
                nc.vector.tensor_scalar(
                    out=invs2[:], in0=y2[:], scalar1=m_wd_col[:],
                    scalar2=1.0 / (127.0 * float(H) * D),
                    op0=OP.mult, op1=OP.mult)
                r2 = sbCol.tile([128, SEG], f32, tag="st_i")
                nc.vector.reciprocal(r2[:], y2[:])
                alpha = sbCol.tile([128, SEG], f32, tag="st_j")
                nc.vector.tensor_tensor(out=alpha[:], in0=r2[:], in1=rs[:],
                                        op=OP.mult)
                alpha2 = sbCol.tile([128, SEG], f32, tag="st_k")
                nc.vector.tensor_scalar(out=alpha2[:], in0=alpha[:],
                                        scalar1=127.0, scalar2=None,
                                        op0=OP.mult)
                for i in range(SEG):
                    t = t0 + i
                    w1 = sbT1.tile([128, FS], f32, tag="T1")
                    # requant magic-add on scalar (keeps vector free for the
                    # rowsum); the -MAGIC step stays on scalar too
                    nc.scalar.activation(out=w1[:], in_=ht_tiles[t][:],
                                         func=AF.Copy,
                                         scale=alpha2[:, i:i + 1], bias=MAGIC)
                    hq = sbT1.tile([128, FS], f32, tag="T1")
                    nc.scalar.activation(out=hq[:], in_=w1[:],
                                         func=AF.Identity, bias=negmagic[:])
                    qacc = sbCol.tile([128, 1], f32, tag="qacc")
                    mult_rowsum(hq[:], Sh_bcast[:], qacc, None)
                    nc.vector.tensor_scalar(out=Q_cols[:, t:t + 1],
                                            in0=qacc[:],
                                            scalar1=invs2[:, i:i + 1],
                                            scalar2=None, op0=OP.mult)

            # ================= main matmul loop =================
            with tc.tile_pool(name="psM", bufs=2, space="PSUM") as psM:
                nxt_xq = None
                for gi, (t0, gsz) in enumerate(GROUPS):
                    for tl in range(gsz):
                        t = t0 + tl
                        tc0 = tl * 128
                        gps = [psM.tile([128, NF], f32, tag=f"g{j}",
                                        name=f"gp{j}") for j in range(FH)]
                        ups = [psM.tile([128, NF], f32, tag=f"u{j}",
                                        name=f"up{j}") for j in range(FH)]
                        for di, d in enumerate(D_ORDER):
                            lhsT = cur_xq[d][:, tc0:tc0 + 128]
                            s0, s1 = (di == 0), (di == DT - 1)
                            for j in range(FH):
                                nc.tensor.matmul(gps[j][:], lhsT,
                                                 wq_g[d][:, j * NF:(j + 1) * NF],
                                                 start=s0, stop=s1)
                                nc.tensor.matmul(ups[j][:], lhsT,
                                                 wq_u[d][:, j * NF:(j + 1) * NF],
                                                 start=s0, stop=s1)
                        us = sbUG.tile([128, FS], fp16, tag="us")
                        gsl = sbUG.tile([128, FS], fp16, tag="gs")
                        for j in range(FH):
                            nc.scalar.activation(out=us[:, j * NF:(j + 1) * NF],
                                                 in_=ups[j][:], func=AF.Copy,
                                                 scale=DEQU[:, t:t + 1])
                            nc.scalar.activation(out=gsl[:, j * NF:(j + 1) * NF],
                                                 in_=gps[j][:], func=AF.Silu,
                                                 scale=DEQG[:, t:t + 1])
                        ht = sbH.tile([128, FS], fp16, tag="h")
                        ht_tiles[t] = ht
                        if ln_is_ones:
                            nc.vector.tensor_tensor(out=ht[:], in0=gsl[:],
                                                    in1=us[:], op=OP.mult)
                            hsq = sbUG.tile([128, FS], fp16, tag="hsq", bufs=1)
                            nc.scalar.activation(
                                out=hsq[:], in_=ht[:], func=AF.Square,
                                accum_out=ssq_cols[:, t:t + 1])
                            nc.vector.tensor_reduce(
                                out=am_cols[:, t:t + 1], in_=ht[:], axis=AX.X,
                                op=OP.max, apply_absolute_value=True)
                        else:
                            htf = sbT1.tile([128, FS], f32, tag="T1")
                            nc.vector.tensor_tensor(out=htf[:], in0=gsl[:],
                                                    in1=us[:], op=OP.mult)
                            hsq = sbUG.tile([128, FS], fp16, tag="hsq", bufs=1)
                            nc.scalar.activation(
                                out=hsq[:], in_=htf[:], func=AF.Square,
                                accum_out=ssq_cols[:, t:t + 1])
                            nc.vector.tensor_tensor(out=ht[:], in0=htf[:],
                                                    in1=Ln_bcast[:],
                                                    op=OP.mult)
                            nc.vector.tensor_reduce(
                                out=am_cols[:, t:t + 1], in_=ht[:], axis=AX.X,
                                op=OP.max, apply_absolute_value=True)
                        # segment boundary: slot stats, one AllReduce(max)
                        for s in range(NSEG):
                            if t == SEGB[s + 1] - 1:
                                a, b2 = SEGB[s], SEGB[s + 1]
                                seg = b2 - a
                                mrep = mask_bcast[:].unsqueeze(1) \
                                    .broadcast_to([128, seg, NCORES])
                                for ci, cols in enumerate((ssq_cols, am_cols)):
                                    slt = sbCol.tile([128, seg * NCORES], f32,
                                                     tag=f"slt{ci}",
                                                     name=f"slt{ci}")
                                    nc.vector.tensor_tensor(
                                        out=slt[:].rearrange(
                                            "p (i r) -> p i r", r=NCORES),
                                        in0=cols[:, a:b2].unsqueeze(2)
                                        .broadcast_to([128, seg, NCORES]),
                                        in1=mrep, op=OP.mult)
                                    nc.sync.dma_start(
                                        st_in[s][128 * ci:128 * (ci + 1), :],
                                        slt[:])
                                collective("AllReduce", OP.max,
                                           st_in[s][:], st_out[s][:])
                        # staggered Wd stat pass: the |.| stat for slab i
                        # runs two tiles after its DMA was emitted, so the
                        # 2-buf ring stays acyclic with loop prefetches
                        if 7 <= t <= 14:
                            emit_wd_stats(2 * (t - 7), 2 * (t - 7) + 2)
                        if 5 <= t <= 12:
                            emit_wd_dma(2 * (t - 5))
                            emit_wd_dma(2 * (t - 5) + 1)
                        if gi == 4 and tl == 0:
                            emit_c2_scalars()
                        if gi == 4 and tl >= 1:
                            emit_wd_chain(3 * (tl - 1), min(3 * tl, 8))
                        # prefetch next group's xq after the 2nd tile
                        if tl == min(1, gsz - 1) and gi + 1 < len(GROUPS):
                            nxt_xq = emit_xq(gi + 1, [None] * DT)
                    if gi == 3:
                        emit_c2()
                    if gi == 4:
                        emit_sh()
                    if POST_AT.get(gi):
                        emit_post(POST_AT[gi])
                    if gi + 1 < len(GROUPS):
                        cur_xq, nxt_xq = nxt_xq, None

            if POST_TAIL:
                emit_post(POST_TAIL)

            # ============ pooled partials + classifier ============
            with tc.tile_pool(name="psE", bufs=1, space="PSUM") as psE:
                pq = psE.tile([1, TT], f32, tag="pq")
                nc.tensor.matmul(pq[:], ones_col[:], Q_cols[:],
                                 start=True, stop=True)
                plrow = sbC.tile([1, TT], f32)
                nc.vector.tensor_copy(plrow[:], pq[:])
                nc.sync.dma_start(pl_in[:], plrow[:])
                collective("AllReduce", OP.add, pl_in[:], pl_out[:])

                pool3 = sbC.tile([C, B], f32)
                nc.sync.dma_start(
                    pool3[:], pl_out[:].rearrange("o (b c) -> (o c) b", c=C))
                clsW_sb = sbC.tile([C, NCLS], f32)
                nc.sync.dma_start(clsW_sb[:], clsWT_t.ap())
                clsb_sb = sbC.tile([1, NCLS], f32)
                nc.sync.dma_start(clsb_sb[:], clsb_t.ap())
                out_sb = sbC.tile([B, NCLS], f32)
                for j in range(0, NCLS, 512):
                    w = min(512, NCLS - j)
                    pcls = psE.tile([B, 512], f32, tag="pcls", bufs=2)
                    nc.tensor.matmul(pcls[:, 0:w], pool3[:],
                                     clsW_sb[:, j:j + w], start=True,
                                     stop=False)
                    nc.tensor.matmul(pcls[:, 0:w], ones1[:, 0:B],
                                     clsb_sb[:, j:j + w], start=False,
                                     stop=True)
                    nc.vector.tensor_copy(out_sb[:, j:j + w], pcls[:, 0:w])
                nc.sync.dma_start(out_t.ap(), out_sb[:])

    nc.compile()
    meta = dict(B=B, C=C, H=H, D=D, F=F, NCLS=NCLS, NCORES=NCORES,
                T=T, TS=TS, FS=FS)
    return nc, meta


def make_in_maps(x, Wg, Wu, Wd, ln_w, cls_W, cls_b, meta):
    """Host-side sharding: slices/transposes only, no arithmetic."""
    T, TS, FS = meta["T"], meta["TS"], meta["FS"]
    D = meta["D"]
    NCLS = meta["NCLS"]
    NCORES = meta["NCORES"]
    xf = np.ascontiguousarray(np.asarray(x, np.float32).reshape(T, D))
    xT = np.ascontiguousarray(xf.T)
    clsWT = np.ascontiguousarray(np.asarray(cls_W, np.float32).T)
    clsb2 = np.ascontiguousarray(np.asarray(cls_b, np.float32).reshape(1, NCLS))
    maps = []
    for k in range(NCORES):
        f0 = k * FS
        cmask = np.zeros((1, NCORES), np.float32)
        cmask[0, k] = 1.0
        maps.append({
            "xs": np.ascontiguousarray(xf[k * TS:(k + 1) * TS]),
            "xT": xT,
            "wgT": np.ascontiguousarray(np.asarray(Wg, np.float32)[f0:f0 + FS, :].T),
            "wuT": np.ascontiguousarray(np.asarray(Wu, np.float32)[f0:f0 + FS, :].T),
            "wdT": np.ascontiguousarray(np.asarray(Wd, np.float32)[:, f0:f0 + FS].T),
            "lnw": np.ascontiguousarray(np.asarray(ln_w, np.float32)[f0:f0 + FS].reshape(1, FS)),
            "clsWT": clsWT,
            "clsb": clsb2,
            "cmask": cmask,
        })
    return maps


_CACHE = {}


def kernel(x, Wg, Wu, Wd, ln_w, cls_W, cls_b):
    """Takes FULL inputs, runs the 8-core SPMD Bass kernel, returns [B, NCLS]."""
    from concourse import bass_utils

    x = np.asarray(x, np.float32)
    B, C, H, D = x.shape
    F = int(np.asarray(Wg).shape[0])
    NCLS = int(np.asarray(cls_W).shape[0])
    ln_ones = bool(np.all(np.asarray(ln_w) == 1.0))
    key = (B, C, H, D, F, NCLS, ln_ones)
    if key not in _CACHE:
        _CACHE[key] = build(B=B, C=C, H=H, D=D, F=F, NCLS=NCLS, NCORES=8,
                            ln_is_ones=ln_ones)
    nc, meta = _CACHE[key]
    in_maps = make_in_maps(x, Wg, Wu, Wd, ln_w, cls_W, cls_b, meta)
    res = bass_utils.run_bass_kernel_spmd(nc, in_maps, core_ids=list(range(8)))
    return np.asarray(res.results[0]["out"], np.float32)


# revision 59
# speedup vs baseline: 1.0250x; 1.0089x over previous
"""BitNet SwiGLU MLP kernel for Trainium2, tensor-parallel over 8 NeuronCores.

Sharding (Megatron-style TP over the intermediate dim F):
- Each core holds a 1/8 column-shard of Wg/Wu (fed transposed: [D, FS]) and
  the matching shard of Wd (fed as Wd[:, shard].T = [FS, D]). x is replicated,
  fed both natural-sliced (per-token quant stats, sharded over tokens) and
  fully transposed [D, T] (matmul operand layout).
- bit_linear runs as an exact integer matmul: quantized activations are ints
  in [-128,127] (bf16 lhsT) and ternary weights in {-1,0,1} (fp8e4 rhs, both
  exact, accumulated exactly in fp32 PSUM), dequantized on the output by
  per-token / global scales. clip(round(.)) for activations is exact RNE via
  the fp32 magic-number trick (the clip never binds since |x*scale| <= 127).
- Down-proj + mean-pool is collapsed algebraically:
  mean_{h,d}(hq @ Wdq.T) = 1/(H*D) * sum_f hq[t,f] * S[f],  S = colsum(Wdq)
  so only a per-token weighted row-reduction against S remains.
- h is kept resident in SBUF as fp16 between the main loop and the
  per-token requantization pass; no DRAM roundtrip.
- Head is latency-optimized: weight |.| stats stream dual-engine
  (vector+scalar) as DMA lands, cross-partition sums go through gpsimd (the
  PE never sits on the c1 critical path), the quant-pass re-read is
  prefetched during the c1 AllReduce wait, and the Wd stat/quant work is
  deferred into the early loop groups where engines have slack.
- Per-segment RMS/absmax stats cross cores via ONE AllReduce(max) with
  per-core slots (mask built from a one-hot input); pooled partials are
  AllReduced at the end; every core runs the tiny classifier.
"""
import numpy as np

MAGIC = 12582912.0  # 1.5 * 2^23, fp32 RNE magic
EPS = 1e-6
QEPS = 1e-5


def build(B=8, C=3, H=128, D=2048, F=8192, NCLS=1000, NCORES=8,
          ln_is_ones=True, mock_collectives=False,
          NDUM1=480, NDUM2=55, H_BUFS=20, XQ_BUFS=20, RQ_BUFS=2,
          SAFE_CLIP=False, SAFE_BCAST=True, SAFE_TTR=True):
    """Build + compile the SPMD Bass program. Returns (nc, meta)."""
    import concourse.bacc as bacc
    import concourse.tile as tile
    from concourse import mybir
    from concourse import bass_isa

    f32 = mybir.dt.float32
    bf16 = mybir.dt.bfloat16
    fp16 = mybir.dt.float16
    fp8 = mybir.dt.float8e4
    AX = mybir.AxisListType
    OP = mybir.AluOpType
    AF = mybir.ActivationFunctionType
    RO = bass_isa.ReduceOp
    RG = [list(range(NCORES))]

    assert H == 128
    T = B * C * H
    TT = T // 128               # token tiles (== B*C) = 24
    TS = T // NCORES            # tokens per core for x stats
    TST = TS // 128
    FS = F // NCORES            # f-shard width = 1024
    DT = D // 128               # contraction tiles = 16
    NF = 512
    FH = FS // NF               # = 2
    WB = 2                      # d-tiles per weight/x DMA slab

    # token-tile groups (last ones smaller to tighten the tail)
    GROUPS = [(0, 4), (4, 4), (8, 4), (12, 4), (16, 4), (20, 2), (22, 2)]
    assert sum(g[1] for g in GROUPS) == TT
    SEGB = [0, 4, 8, 12, 16, 20, 23, 24]
    NSEG = len(SEGB) - 1
    def group_of(t):
        for gi, (t0, gsz) in enumerate(GROUPS):
            if t0 <= t < t0 + gsz:
                return gi
        raise AssertionError
    # post-stats for segment s run AFTER group gpost's tiles (so an engine
    # FIFO wait on the segment's collective can never block later tiles'
    # compute that feeds later collectives). Early segments get extra slack
    # because Sh_bcast (built from the in-loop Wd quant) lands around g3.
    # Sh_bcast (from the in-loop Wd quant chain) lands at the end of group
    # 4, so the early segments all post right after it; h tiles are fully
    # SBUF-resident so late posts never throttle the loop.
    POST_AT = {4: [0, 1, 2], 5: [3], 6: [4, 5]}
    POST_TAIL = [6]

    nc = bacc.Bacc("TRN2", target_bir_lowering=False, debug=False,
                   num_devices=1 if mock_collectives else NCORES)

    def collective(kind, op, in_ap, out_ap):
        if NCORES == 1 or mock_collectives:
            n = out_ap.size() // in_ap.size()
            flat = out_ap.rearrange("a b -> (a b)")
            for r in range(n):
                nc.sync.dma_start(
                    flat[r * in_ap.size():(r + 1) * in_ap.size()], in_ap)
        else:
            nc.gpsimd.collective_compute(kind, op, replica_groups=RG,
                                         ins=[in_ap.opt()], outs=[out_ap.opt()])

    xs_t = nc.dram_tensor("xs", [TS, D], f32, kind="ExternalInput")
    xT_t = nc.dram_tensor("xT", [D, T], f32, kind="ExternalInput")
    wgT_t = nc.dram_tensor("wgT", [D, FS], f32, kind="ExternalInput")
    wuT_t = nc.dram_tensor("wuT", [D, FS], f32, kind="ExternalInput")
    wdT_t = nc.dram_tensor("wdT", [FS, D], f32, kind="ExternalInput")
    lnw_t = nc.dram_tensor("lnw", [1, FS], f32, kind="ExternalInput")
    clsWT_t = nc.dram_tensor("clsWT", [C, NCLS], f32, kind="ExternalInput")
    clsb_t = nc.dram_tensor("clsb", [1, NCLS], f32, kind="ExternalInput")
    cmask_t = nc.dram_tensor("cmask", [1, NCORES], f32, kind="ExternalInput")
    out_t = nc.dram_tensor("out", [B, NCLS], f32, kind="ExternalOutput")

    def r128(ap):
        # [1, n*128] dram view -> [128, n] (partition = fast axis)
        return ap.rearrange("o (i p) -> (o p) i", p=128)

    with tile.TileContext(nc) as tc:
        import contextlib
        with contextlib.ExitStack() as st:
            dram = st.enter_context(tc.tile_pool(name="dram", bufs=1, space="DRAM"))
            sbC = st.enter_context(tc.tile_pool(name="sbC", bufs=1))
            sbS8 = st.enter_context(tc.tile_pool(name="sbS8", bufs=2))
            sbT1 = st.enter_context(tc.tile_pool(name="sbT1", bufs=2))
            sbS2 = st.enter_context(tc.tile_pool(name="sbS2", bufs=3))
            sbUG = st.enter_context(tc.tile_pool(name="sbUG", bufs=2))
            sbCol = st.enter_context(tc.tile_pool(name="sbCol", bufs=4))
            sbH = st.enter_context(tc.tile_pool(name="sbH", bufs=H_BUFS))
            sbXQ = st.enter_context(tc.tile_pool(name="sbXQ", bufs=XQ_BUFS))
            sbWQ = st.enter_context(tc.tile_pool(name="sbWQ", bufs=2 * DT))

            sc_in = dram.tile([1, TS], f32)
            sc_out = dram.tile([1, T], f32)
            c1_in = dram.tile([1, 8], f32)
            c1_out = dram.tile([1, 8], f32)
            c2_in = dram.tile([1, 8], f32)
            c2_out = dram.tile([1, 8], f32)
            srow_dram = dram.tile([1, FS], f32)
            srow2_dram = dram.tile([1, T], f32)
            dum_dram = dram.tile([1, 8], f32)
            # slotted stats exchange: one AllReduce(max) per segment over
            # [256, SEG*8]; core k's ssq/am partials sit in slot k of the
            # innermost axis (all other slots zero, and partials are >= 0,
            # so max == gather). Local free-axis reduce then combines slots.
            st_in = [dram.tile([256, (SEGB[s + 1] - SEGB[s]) * NCORES], f32,
                               name=f"st_in{s}") for s in range(NSEG)]
            st_out = [dram.tile([256, (SEGB[s + 1] - SEGB[s]) * NCORES], f32,
                                name=f"st_out{s}", addr_space="Shared")
                      for s in range(NSEG)]
            pl_in = dram.tile([1, TT], f32)
            pl_out = dram.tile([1, TT], f32, addr_space="Shared")

            ones1 = sbC.tile([1, 128], f32)
            nc.vector.memset(ones1[:], 1.0)
            ones_col = sbC.tile([128, 1], f32)
            nc.vector.memset(ones_col[:], 1.0)
            negmagic = sbC.tile([128, 1], f32)
            nc.vector.memset(negmagic[:], -MAGIC)
            zeros_bf = sbC.tile([128, NF], bf16)
            nc.vector.memset(zeros_bf[:], 0.0)

            wacc = sbC.tile([128, 48], f32)
            mask_bcast = sbC.tile([128, NCORES], f32)
            sc_cols = sbC.tile([128, TST], f32)
            c1_sb = sbC.tile([1, 8], f32)
            c2_sb = sbC.tile([1, 8], f32)
            m_w_col = sbC.tile([128, 2], f32)
            s_w_col = sbC.tile([128, 2], f32)
            m_wd_col = sbC.tile([128, 1], f32)
            s_wd_col = sbC.tile([128, 1], f32)
            S_bcast = sbC.tile([128, T], f32)
            Sh_bcast = sbC.tile([128, FS], f32)
            DEQG = sbC.tile([128, TT], f32)
            DEQU = sbC.tile([128, TT], f32)
            ssq_cols = sbC.tile([128, TT], f32)
            am_cols = sbC.tile([128, TT], f32)
            Q_cols = sbC.tile([128, TT], f32)
            Ssh_cols = sbC.tile([128, FS // 128], f32)
            if not ln_is_ones:
                Ln_bcast = sbC.tile([128, FS], f32)

            ht_tiles = [None] * TT

            def bcast_row(out_cols, in_row, n):
                # out_cols [128, n] <- broadcast of in_row [1, n]
                if SAFE_BCAST:
                    pad = sbCol.tile([128, max(n, 1)], f32, tag="bc",
                                     bufs=2, name="bcpad")
                    nc.vector.memset(pad[:, 0:n], 0.0)
                    nc.vector.tensor_copy(pad[0:1, 0:n], in_row[0:1, 0:n])
                    nc.gpsimd.partition_all_reduce(out_cols[:, 0:n],
                                                   pad[:, 0:n], channels=128,
                                                   reduce_op=RO.add)
                else:
                    nc.gpsimd.partition_broadcast(out_cols[:, 0:n],
                                                  in_row[0:1, 0:n],
                                                  channels=128)

            def clip_step(t1):
                # clamp t1 (rounded magic form) to [MAGIC-1, MAGIC+1]
                if SAFE_CLIP:
                    t2 = sbT1.tile([128, FS], f32, tag="T2", bufs=2)
                    nc.vector.tensor_scalar(out=t2[:], in0=t1[:],
                                            scalar1=MAGIC - 1.0,
                                            scalar2=MAGIC + 1.0,
                                            op0=OP.max, op1=OP.min)
                    return t2
                nc.vector.tensor_scalar(out=t1[:], in0=t1[:],
                                        scalar1=MAGIC - 1.0,
                                        scalar2=MAGIC + 1.0,
                                        op0=OP.max, op1=OP.min)
                return t1

            def mult_rowsum(in0, in1, accum, init):
                # accum [128,1] = init + sum_cols(in0 * in1)
                if SAFE_TTR:
                    jk = sbUG.tile([128, FS], bf16, tag="jkf", bufs=2,
                                   name="jkf")
                    nc.vector.tensor_tensor(out=jk[:], in0=in0, in1=in1,
                                            op=OP.mult)
                    if init is None:
                        nc.vector.tensor_reduce(out=accum[:], in_=jk[:],
                                                axis=AX.X, op=OP.add)
                    else:
                        part = sbCol.tile([128, 1], f32, tag="mrs",
                                          name="mrs")
                        nc.vector.tensor_reduce(out=part[:], in_=jk[:],
                                                axis=AX.X, op=OP.add)
                        nc.vector.tensor_tensor(out=accum[:], in0=part[:],
                                                in1=init[:], op=OP.add)
                else:
                    jk = sbUG.tile([128, FS], bf16, tag="jk", bufs=2,
                                   name="jk")
                    nc.vector.tensor_tensor_reduce(
                        out=jk[:], in0=in0, in1=in1, scale=1.0,
                        scalar=(0.0 if init is None else init[:]),
                        op0=OP.mult, op1=OP.add, accum_out=accum[:])

            def abs_stat(wt, wcol, on_scalar):
                # wcol [128,1] = sum_cols |wt|, on either engine
                if on_scalar:
                    # the elementwise output is never read (fp8 scratch);
                    # only the fp32 row-accumulator matters
                    wsc = sbT1.tile([128, wt.free_size()], fp8, tag="wsc",
                                    bufs=2, name="wsc")
                    nc.scalar.activation(out=wsc[:], in_=wt, func=AF.Abs,
                                         accum_out=wcol)
                else:
                    nc.vector.tensor_reduce(out=wcol, in_=wt, axis=AX.X,
                                            op=OP.add,
                                            apply_absolute_value=True)

            # emit_xq defined up-front: the head emits group 0 itself
            def emit_xq(gi, xq_slabs):
                t0, gsz = GROUPS[gi]
                gw = gsz * 128
                tc0 = t0 * 128
                for d0 in range(0, DT, WB):
                    xsl = sbXQ.tile([128, WB * 512], f32, tag="xsl", bufs=2)
                    nc.sync.dma_start(
                        xsl[:, 0:WB * gw],
                        xT_t.ap()[d0 * 128:(d0 + WB) * 128, tc0:tc0 + gw]
                        .rearrange("(b p) c -> p b c", p=128))
                    for b in range(WB):
                        xp = sbXQ.tile([128, 512], f32, tag="xp", bufs=2)
                        nc.vector.tensor_tensor(
                            out=xp[:, 0:gw], in0=xsl[:, b * gw:(b + 1) * gw],
                            in1=S_bcast[:, tc0:tc0 + gw], op=OP.mult)
                        xq = sbXQ.tile([128, 512], bf16, tag="xq")
                        nc.vector.tensor_scalar(out=xq[:, 0:gw],
                                                in0=xp[:, 0:gw],
                                                scalar1=MAGIC, scalar2=MAGIC,
                                                op0=OP.add, op1=OP.subtract)
                        xq_slabs[d0 + b] = xq
                return xq_slabs

            # ================= Head =================
            wd_tiles = [None] * (2 * (FS // 128))
            with tc.tile_pool(name="psH", bufs=1, space="PSUM") as psH:
                # dummy matmuls keep the PE p-state up until real MMs arrive
                if NDUM1 > 0:
                    dum = psH.tile([128, NF], f32, tag="dum")
                    for k in range(NDUM1):
                        nc.tensor.matmul(dum[:], zeros_bf[:, 0:128],
                                         zeros_bf[:], start=(k == 0),
                                         stop=(k == NDUM1 - 1))

                # x per-token clipped absmax -> sc_in -> AllGather (first
                # collective; also warms the CC stream behind the barrier).
                # Halved DMAs ride both hardware DGE queues concurrently.
                for i in range(TST):
                    hs = []
                    for h2 in range(2):
                        xt = sbS8.tile([128, FS], f32, bufs=4,
                                       tag=("Sg" if h2 == 0 else "Su"))
                        nc.gpsimd.dma_start(
                            xt[:], xs_t.ap()[i * 128:(i + 1) * 128,
                                             h2 * FS:(h2 + 1) * FS])
                        hs.append(xt)
                    am = sbCol.tile([128, 2], f32, tag="am")
                    for h2 in range(2):
                        nc.vector.tensor_reduce(out=am[:, h2:h2 + 1],
                                                in_=hs[h2][:], axis=AX.X,
                                                op=OP.max,
                                                apply_absolute_value=True)
                    am1 = sbCol.tile([128, 1], f32, tag="am1")
                    nc.vector.tensor_reduce(out=am1[:], in_=am[:], axis=AX.X,
                                            op=OP.max)
                    nc.vector.tensor_scalar(out=sc_cols[:, i:i + 1], in0=am1[:],
                                            scalar1=QEPS, scalar2=None,
                                            op0=OP.max)
                nc.sync.dma_start(r128(sc_in[:]), sc_cols[:])
                collective("AllGather", OP.bypass, sc_in[:], sc_out[:])

                # Wg/Wu |.| sums. Two hardware DGE queues run concurrently:
                # g slabs load via SP (reduce on vector), u slabs via the
                # Activation HWDGE (reduce on scalar). Stats lag their slab
                # by one iteration so neither engine FIFO blocks the other
                # queue's triggers.
                stat_q = []
                stat_wts = {}
                for d in range(DT):
                    wts = []
                    for j, ten in enumerate((wgT_t, wuT_t)):
                        eng = nc.sync if j == 0 else nc.scalar
                        wt = sbS8.tile([128, FS], f32, bufs=4,
                                       tag=("Sg" if j == 0 else "Su"))
                        eng.dma_start(wt[:],
                                      ten.ap()[d * 128:(d + 1) * 128, :])
                        wts.append(wt)
                    stat_wts[d] = wts
                    stat_q.append((d, wts))
                    if len(stat_q) > 1:
                        pi, pw = stat_q.pop(0)
                        for j in range(2):
                            abs_stat(pw[j][:], wacc[:, j * 16 + pi:j * 16 + pi + 1],
                                     on_scalar=False)
                for pi, pw in stat_q:
                    for j in range(2):
                        abs_stat(pw[j][:], wacc[:, j * 16 + pi:j * 16 + pi + 1],
                                 on_scalar=False)
                # cross-partition totals via gpsimd (PE stays off this path)
                colgu = sbCol.tile([128, 2], f32, tag="cgu")
                nc.vector.tensor_reduce(out=colgu[:, 0:1], in_=wacc[:, 0:16],
                                        axis=AX.X, op=OP.add)
                nc.vector.tensor_reduce(out=colgu[:, 1:2], in_=wacc[:, 16:32],
                                        axis=AX.X, op=OP.add)
                c1gu = sbCol.tile([128, 2], f32, tag="c1gu")
                nc.gpsimd.partition_all_reduce(c1gu[:], colgu[:], channels=128,
                                               reduce_op=RO.add)
                nc.vector.memset(c1_sb[:], 0.0)
                nc.vector.tensor_copy(c1_sb[:, 0:2], c1gu[0:1, 0:2])
                nc.sync.dma_start(c1_in[:], c1_sb[:])
                collective("AllReduce", OP.add, c1_in[:], c1_out[:])

                # S_bcast = 127 / absmax: the reciprocal runs on the
                # compact [128, TT] gathered form (DVE reciprocal is slow),
                # then a DRAM bounce re-rows it and the PE broadcasts it
                # down partitions. Emitted before emit_xq (vector FIFO
                # ordering). Numerically identical to reciprocal-then-scale
                # on the broadcast form.
                yraw = sbC.tile([128, TT], f32)
                nc.gpsimd.dma_start(yraw[:], r128(sc_out[:]))
                r24 = sbC.tile([128, TT], f32)
                nc.vector.reciprocal(r24[:], yraw[:])
                s24 = sbC.tile([128, TT], f32)
                nc.vector.tensor_scalar(out=s24[:], in0=r24[:],
                                        scalar1=127.0, scalar2=None,
                                        op0=OP.mult)
                nc.sync.dma_start(r128(srow2_dram[:]), s24[:])
                for j in range(0, T, 512):
                    scr = sbS2.tile([1, 512], f32, tag="scr")
                    nc.gpsimd.dma_start(scr[:], srow2_dram[0:1, j:j + 512])
                    pb = psH.tile([128, 512], f32, tag="pb", bufs=2)
                    nc.tensor.matmul(pb[:], ones1[:], scr[:],
                                     start=True, stop=True)
                    nc.scalar.activation(out=S_bcast[:, j:j + 512],
                                         in_=pb[:], func=AF.Copy)

                # group-0 xq: xT DMAs queue right behind the stat reads
                cur_xq = emit_xq(0, [None] * DT)

                # ---- derived scalars from c1 (g,u) ----
                # (these DMAs are emitted BEFORE the rq stream below so no
                # rq slab -- whose buffer release depends on them -- can sit
                # ahead of them in a DMA queue: that would deadlock)
                c1g = sbC.tile([1, 8], f32)
                nc.gpsimd.dma_start(c1g[:], c1_out[:])
                m_w = sbC.tile([1, 2], f32)
                nc.vector.tensor_scalar(out=m_w[:], in0=c1g[:, 0:2],
                                        scalar1=1.0 / (float(F) * D),
                                        scalar2=QEPS, op0=OP.mult, op1=OP.max)
                s_w = sbC.tile([1, 2], f32)
                nc.vector.reciprocal(s_w[:], m_w[:])
                bcast_row(m_w_col, m_w, 2)
                bcast_row(s_w_col, s_w, 2)

                nc.vector.tensor_scalar(out=DEQG[:], in0=yraw[:],
                                        scalar1=m_w_col[:, 0:1],
                                        scalar2=1.0 / 127.0,
                                        op0=OP.mult, op1=OP.mult)
                nc.vector.tensor_scalar(out=DEQU[:], in0=yraw[:],
                                        scalar1=m_w_col[:, 1:2],
                                        scalar2=1.0 / 127.0,
                                        op0=OP.mult, op1=OP.mult)

                # per-core one-hot slot mask (c1-wait window)
                cm_row = sbC.tile([1, NCORES], f32)
                nc.sync.dma_start(cm_row[:], cmask_t.ap())
                bcast_row(mask_bcast, cm_row, NCORES)

                # classifier constants preloaded off the tail-critical path
                clsW_sb = sbC.tile([C, NCLS], f32)
                nc.gpsimd.dma_start(clsW_sb[:], clsWT_t.ap())
                clsb_sb = sbC.tile([1, NCLS], f32)
                nc.gpsimd.dma_start(clsb_sb[:], clsb_t.ap())

                # second dummy batch bridges the gap to the first real MMs
                if NDUM1 > 0 and NDUM2 > 0:
                    dum2 = psH.tile([128, NF], f32, tag="dum2")
                    for k in range(NDUM2):
                        nc.tensor.matmul(dum2[:], zeros_bf[:, 0:128],
                                         zeros_bf[:], start=(k == 0),
                                         stop=(k == NDUM2 - 1))
                    dsc = sbC.tile([1, 8], f32)
                    nc.vector.memset(dsc[:], 0.0)
                    nc.vector.tensor_copy(dsc[:, 0:1], dum[0:1, 0:1])
                    nc.vector.tensor_copy(dsc[:, 1:2], dum2[0:1, 0:1])
                    nc.sync.dma_start(dum_dram[:], dsc[:])

                if not ln_is_ones:
                    if SAFE_BCAST:
                        lnpad = sbT1.tile([128, FS], f32, tag="T1")
                        nc.vector.memset(lnpad[:], 0.0)
                        nc.sync.dma_start(lnpad[0:1, :], lnw_t.ap())
                        nc.gpsimd.partition_all_reduce(
                            Ln_bcast[:], lnpad[:], channels=128,
                            reduce_op=RO.add)
                    else:
                        lnr = sbC.tile([1, FS], f32)
                        nc.sync.dma_start(lnr[:], lnw_t.ap())
                        nc.gpsimd.partition_broadcast(Ln_bcast[:], lnr[:],
                                                      channels=128)

            # ================= Wg/Wu quantization stream =================
            # The last RESID stat pairs are still live in the 4-deep rings,
            # so the chain starts on them at c1 while the other 12 pairs
            # re-read; each re-read is emitted right after the chain step
            # whose ring slot it reuses, so the ring semaphores self-pace
            # the stream with prefetch distance 4 (no FIFO cycles).
            RESID = 4
            D_ORDER = list(range(DT - RESID, DT)) + list(range(DT - RESID))
            wq_g = [None] * DT
            wq_u = [None] * DT
            rrtiles = {}
            for j, dd in enumerate(D_ORDER):
                wts = stat_wts[dd] if j < RESID else rrtiles.pop(dd)
                for lst, scol in ((wq_g, 0), (wq_u, 1)):
                    wt = wts[scol]
                    t1 = sbT1.tile([128, FS], f32, tag="T1")
                    if scol == 0:
                        nc.scalar.activation(out=t1[:], in_=wt[:],
                                             func=AF.Copy,
                                             scale=s_w_col[:, scol:scol + 1],
                                             bias=MAGIC)
                    else:
                        nc.vector.tensor_scalar(
                            out=t1[:], in0=wt[:],
                            scalar1=s_w_col[:, scol:scol + 1],
                            scalar2=MAGIC, op0=OP.mult, op1=OP.add)
                    t2 = clip_step(t1)
                    wq = sbWQ.tile([128, FS], fp8, tag="wq")
                    nc.scalar.activation(out=wq[:], in_=t2[:],
                                         func=AF.Copy, bias=-MAGIC)
                    lst[dd] = wq
                if j + RESID < DT:
                    nd = D_ORDER[j + RESID]
                    pair = []
                    for jj, ten in enumerate((wgT_t, wuT_t)):
                        eng = nc.sync if jj == 0 else nc.scalar
                        wt = sbS8.tile([128, FS], f32, bufs=4,
                                       tag=("Sg" if jj == 0 else "Su"))
                        eng.dma_start(wt[:],
                                      ten.ap()[nd * 128:(nd + 1) * 128, :])
                        pair.append(wt)
                    rrtiles[nd] = pair

            # ============ Wd stats/quant helpers (run inside the loop) =====
            def emit_wd_dma(i):
                # half-slab [128 f, 1024 d]; rides the gpsimd DGE ring so it
                # never contends with the quant re-read on the HWDGE queues
                wt = sbS8.tile([128, FS], f32, bufs=4,
                               tag=("Sg" if i % 2 == 0 else "Su"),
                               name=f"wdr{i}")
                nc.gpsimd.dma_start(
                    wt[:], wdT_t.ap()[(i // 2) * 128:(i // 2 + 1) * 128,
                                      (i % 2) * FS:(i % 2 + 1) * FS])
                wd_tiles[i] = wt

            def emit_wd_stats(i0, i1):
                for i in range(i0, i1):
                    abs_stat(wd_tiles[i][:], wacc[:, 32 + i:33 + i],
                             on_scalar=(i % 2 == 1))

            def emit_c2():
                cold = sbCol.tile([128, 1], f32, tag="cd")
                nc.vector.tensor_reduce(out=cold[:], in_=wacc[:, 32:48],
                                        axis=AX.X, op=OP.add)
                c2gu = sbCol.tile([128, 1], f32, tag="c2gu")
                nc.gpsimd.partition_all_reduce(c2gu[:], cold[:], channels=128,
                                               reduce_op=RO.add)
                nc.vector.memset(c2_sb[:], 0.0)
                nc.vector.tensor_copy(c2_sb[:, 0:1], c2gu[0:1, 0:1])
                nc.sync.dma_start(c2_in[:], c2_sb[:])
                collective("AllReduce", OP.add, c2_in[:], c2_out[:])

            def emit_c2_scalars():
                c2g = sbC.tile([1, 8], f32)
                nc.gpsimd.dma_start(c2g[:], c2_out[:])
                mws = sbC.tile([1, 2], f32)
                nc.vector.tensor_scalar(out=mws[:, 0:1], in0=c2g[:, 0:1],
                                        scalar1=1.0 / (float(F) * D),
                                        scalar2=QEPS, op0=OP.mult, op1=OP.max)
                nc.vector.reciprocal(mws[:, 1:2], mws[:, 0:1])
                mwsc = sbC.tile([128, 2], f32)
                bcast_row(mwsc, mws, 2)
                nc.vector.tensor_copy(m_wd_col[:], mwsc[:, 0:1])
                nc.vector.tensor_copy(s_wd_col[:], mwsc[:, 1:2])

            def emit_wd_chain(i0, i1):
                # re-read + quantize Wd slabs, accumulate ternary column sums
                for i in range(i0, i1):
                    chs = []
                    for b in range(2):
                        eng = nc.sync if b == 0 else nc.scalar
                        wt = sbS8.tile([128, FS], f32, bufs=4,
                                       tag=("Sg" if b == 0 else "Su"),
                                       name=f"wdq{i}_{b}")
                        eng.dma_start(
                            wt[:], wdT_t.ap()[i * 128:(i + 1) * 128,
                                              b * FS:(b + 1) * FS])
                        t1 = sbT1.tile([128, FS], f32, tag="T1")
                        if b == 0:
                            nc.scalar.activation(
                                out=t1[:], in_=wt[:],
                                func=AF.Copy, scale=s_wd_col[:], bias=MAGIC)
                        else:
                            nc.vector.tensor_scalar(
                                out=t1[:], in0=wt[:],
                                scalar1=s_wd_col[:], scalar2=MAGIC,
                                op0=OP.mult, op1=OP.add)
                        t2 = clip_step(t1)
                        ch = sbCol.tile([128, 1], f32, tag=f"wdacc{b}")
                        wdq = sbT1.tile([128, FS], fp8, tag="wdq", bufs=2,
                                        name="wdq")
                        nc.scalar.activation(out=wdq[:], in_=t2[:],
                                             func=AF.Copy, bias=-MAGIC,
                                             accum_out=ch[:])
                        chs.append(ch)
                    nc.vector.tensor_tensor(out=Ssh_cols[:, i:i + 1],
                                            in0=chs[0][:], in1=chs[1][:],
                                            op=OP.add)

            def emit_sh():
                # S row -> broadcast down partitions
                nc.sync.dma_start(r128(srow_dram[:]), Ssh_cols[:])
                if SAFE_BCAST:
                    shpad = sbT1.tile([128, FS], f32, tag="T1")
                    nc.vector.memset(shpad[:], 0.0)
                    nc.sync.dma_start(shpad[0:1, :], srow_dram[:])
                    nc.gpsimd.partition_all_reduce(Sh_bcast[:], shpad[:],
                                                   channels=128,
                                                   reduce_op=RO.add)
                else:
                    srow = sbC.tile([1, FS], f32)
                    nc.sync.dma_start(srow[:], srow_dram[:])
                    nc.gpsimd.partition_broadcast(Sh_bcast[:], srow[:],
                                                  channels=128)

            # ================= post-stats (requant h, dot with S) =========
            # handles a LIST of contiguous segments with a single gathered
            # stat chain (the per-op overhead, especially DVE reciprocal,
            # dominates the tiny [128, SEG] math)
            def emit_post(segs):
                t0 = SEGB[segs[0]]
                SEG = SEGB[segs[-1] + 1] - t0
                ssq_g = sbCol.tile([128, SEG * NCORES], f32, tag="st_g1")
                am_g = sbCol.tile([128, SEG * NCORES], f32, tag="st_g2")
                off = 0
                for s in segs:
                    sw = SEGB[s + 1] - SEGB[s]
                    nc.sync.dma_start(
                        ssq_g[:, off * NCORES:(off + sw) * NCORES],
                        st_out[s][0:128, :])
                    nc.sync.dma_start(
                        am_g[:, off * NCORES:(off + sw) * NCORES],
                        st_out[s][128:256, :])
                    off += sw
                ssq12 = sbCol.tile([128, SEG], f32, tag="st_a")
                nc.vector.tensor_reduce(
                    out=ssq12[:],
                    in_=ssq_g[:].rearrange("p (i r) -> p i r", r=NCORES),
                    axis=AX.X, op=OP.add)
                am12 = sbCol.tile([128, SEG], f32, tag="st_b")
                nc.vector.tensor_reduce(
                    out=am12[:],
                    in_=am_g[:].rearrange("p (i r) -> p i r", r=NCORES),
                    axis=AX.X, op=OP.max)
                v = sbCol.tile([128, SEG], f32, tag="st_c")
                nc.vector.tensor_scalar(out=v[:], in0=ssq12[:],
                                        scalar1=1.0 / F, scalar2=EPS,
                                        op0=OP.mult, op1=OP.add)
                sv = sbCol.tile([128, SEG], f32, tag="st_d")
                nc.scalar.activation(out=sv[:], in_=v[:], func=AF.Sqrt)
                rs = sbCol.tile([128, SEG], f32, tag="st_e")
                nc.vector.reciprocal(rs[:], sv[:])
                rg = sbCol.tile([128, SEG], f32, tag="st_f")
                nc.vector.tensor_tensor(out=rg[:], in0=rs[:], in1=am12[:],
                                        op=OP.mult)
                y2 = sbCol.tile([128, SEG], f32, tag="st_g")
                nc.vector.tensor_scalar(out=y2[:], in0=rg[:], scalar1=QEPS,
                                        scalar2=None, op0=OP.max)
                invs2 = sbCol.tile([128, SEG], f32, tag="st_h")
                nc.vector.tensor_scalar(
                    out=invs2[:], in0=y2[:], scalar1=m_wd_col[:],
                    scalar2=1.0 / (127.0 * float(H) * D),
                    op0=OP.mult, op1=OP.mult)
                r2 = sbCol.tile([128, SEG], f32, tag="st_i")
                nc.vector.reciprocal(r2[:], y2[:])
                alpha = sbCol.tile([128, SEG], f32, tag="st_j")
                nc.vector.tensor_tensor(out=alpha[:], in0=r2[:], in1=rs[:],
                                        op=OP.mult)
                alpha2 = sbCol.tile([128, SEG], f32, tag="st_k")
                nc.vector.tensor_scalar(out=alpha2[:], in0=alpha[:],
                                        scalar1=127.0, scalar2=None,
                                        op0=OP.mult)
                for i in range(SEG):
                    t = t0 + i
                    w1 = sbT1.tile([128, FS], f32, tag="T1")
                    # requant magic-add on scalar (keeps vector free for the
                    # rowsum); the -MAGIC step stays on scalar too
                    nc.scalar.activation(out=w1[:], in_=ht_tiles[t][:],
                                         func=AF.Copy,
                                         scale=alpha2[:, i:i + 1], bias=MAGIC)
                    hq = sbT1.tile([128, FS], f32, tag="T1")
                    nc.scalar.activation(out=hq[:], in_=w1[:],
                                         func=AF.Identity, bias=negmagic[:])
                    qacc = sbCol.tile([128, 1], f32, tag="qacc")
                    mult_rowsum(hq[:], Sh_bcast[:], qacc, None)
                    nc.vector.tensor_scalar(out=Q_cols[:, t:t + 1],
                                            in0=qacc[:],
                                            scalar1=invs2[:, i:i + 1],
                                            scalar2=None, op0=OP.mult)

            # ================= main matmul loop =================
            with tc.tile_pool(name="psM", bufs=2, space="PSUM") as psM:
                nxt_xq = None
                for gi, (t0, gsz) in enumerate(GROUPS):
                    for tl in range(gsz):
                        t = t0 + tl
                        tc0 = tl * 128
                        gps = [psM.tile([128, NF], f32, tag=f"g{j}",
                                        name=f"gp{j}") for j in range(FH)]
                        ups = [psM.tile([128, NF], f32, tag=f"u{j}",
                                        name=f"up{j}") for j in range(FH)]
                        for di, d in enumerate(D_ORDER):
                            lhsT = cur_xq[d][:, tc0:tc0 + 128]
                            s0, s1 = (di == 0), (di == DT - 1)
                            for j in range(FH):
                                nc.tensor.matmul(gps[j][:], lhsT,
                                                 wq_g[d][:, j * NF:(j + 1) * NF],
                                                 start=s0, stop=s1)
                                nc.tensor.matmul(ups[j][:], lhsT,
                                                 wq_u[d][:, j * NF:(j + 1) * NF],
                                                 start=s0, stop=s1)
                        us = sbUG.tile([128, FS], fp16, tag="us")
                        gsl = sbUG.tile([128, FS], fp16, tag="gs")
                        for j in range(FH):
                            nc.scalar.activation(out=us[:, j * NF:(j + 1) * NF],
                                                 in_=ups[j][:], func=AF.Copy,
                                                 scale=DEQU[:, t:t + 1])
                            nc.scalar.activation(out=gsl[:, j * NF:(j + 1) * NF],
                                                 in_=gps[j][:], func=AF.Silu,
                                                 scale=DEQG[:, t:t + 1])
                        ht = sbH.tile([128, FS], fp16, tag="h")
                        ht_tiles[t] = ht
                        if ln_is_ones:
                            nc.vector.tensor_tensor(out=ht[:], in0=gsl[:],
                                                    in1=us[:], op=OP.mult)
                            hsq = sbUG.tile([128, FS], fp16, tag="hsq", bufs=1)
                            nc.scalar.activation(
                                out=hsq[:], in_=ht[:], func=AF.Square,
                                accum_out=ssq_cols[:, t:t + 1])
                            nc.vector.tensor_reduce(
                                out=am_cols[:, t:t + 1], in_=ht[:], axis=AX.X,
                                op=OP.max, apply_absolute_value=True)
                        else:
                            htf = sbT1.tile([128, FS], f32, tag="T1")
                            nc.vector.tensor_tensor(out=htf[:], in0=gsl[:],
                                                    in1=us[:], op=OP.mult)
                            hsq = sbUG.tile([128, FS], fp16, tag="hsq", bufs=1)
                            nc.scalar.activation(
                                out=hsq[:], in_=htf[:], func=AF.Square,
                                accum_out=ssq_cols[:, t:t + 1])
                            nc.vector.tensor_tensor(out=ht[:], in0=htf[:],
                                                    in1=Ln_bcast[:],
                                                    op=OP.mult)
                            nc.vector.tensor_reduce(
                                out=am_cols[:, t:t + 1], in_=ht[:], axis=AX.X,
                                op=OP.max, apply_absolute_value=True)
                        # segment boundary: slot stats, one AllReduce(max)
                        for s in range(NSEG):
                            if t == SEGB[s + 1] - 1:
                                a, b2 = SEGB[s], SEGB[s + 1]
                                seg = b2 - a
                                mrep = mask_bcast[:].unsqueeze(1) \
                                    .broadcast_to([128, seg, NCORES])
                                for ci, cols in enumerate((ssq_cols, am_cols)):
                                    slt = sbCol.tile([128, seg * NCORES], f32,
                                                     tag=f"slt{ci}",
                                                     name=f"slt{ci}")
                                    nc.vector.tensor_tensor(
                                        out=slt[:].rearrange(
                                            "p (i r) -> p i r", r=NCORES),
                                        in0=cols[:, a:b2].unsqueeze(2)
                                        .broadcast_to([128, seg, NCORES]),
                                        in1=mrep, op=OP.mult)
                                    nc.sync.dma_start(
                                        st_in[s][128 * ci:128 * (ci + 1), :],
                                        slt[:])
                                collective("AllReduce", OP.max,
                                           st_in[s][:], st_out[s][:])
                        # staggered Wd stat pass: the |.| stat for slab i
                        # runs two tiles after its DMA was emitted, so the
                        # 2-buf ring stays acyclic with loop prefetches
                        if 7 <= t <= 14:
                            emit_wd_stats(2 * (t - 7), 2 * (t - 7) + 2)
                        if 5 <= t <= 12:
                            emit_wd_dma(2 * (t - 5))
                            emit_wd_dma(2 * (t - 5) + 1)
                        if gi == 4 and tl == 0:
                            emit_c2_scalars()
                        if gi == 4 and tl >= 1:
                            emit_wd_chain(3 * (tl - 1), min(3 * tl, 8))
                        # prefetch next group's xq after the 2nd tile
                        if tl == min(1, gsz - 1) and gi + 1 < len(GROUPS):
                            nxt_xq = emit_xq(gi + 1, [None] * DT)
                    if gi == 3:
                        emit_c2()
                    if gi == 4:
                        emit_sh()
                    if POST_AT.get(gi):
                        emit_post(POST_AT[gi])
                    if gi + 1 < len(GROUPS):
                        cur_xq, nxt_xq = nxt_xq, None

            if POST_TAIL:
                emit_post(POST_TAIL)

            # ============ pooled partials + classifier ============
            with tc.tile_pool(name="psE", bufs=1, space="PSUM") as psE:
                pq = psE.tile([1, TT], f32, tag="pq")
                nc.tensor.matmul(pq[:], ones_col[:], Q_cols[:],
                                 start=True, stop=True)
                plrow = sbC.tile([1, TT], f32)
                nc.vector.tensor_copy(plrow[:], pq[:])
                nc.sync.dma_start(pl_in[:], plrow[:])
                collective("AllReduce", OP.add, pl_in[:], pl_out[:])

                pool3 = sbC.tile([C, B], f32)
                nc.sync.dma_start(
                    pool3[:], pl_out[:].rearrange("o (b c) -> (o c) b", c=C))
                out_sb = sbC.tile([B, NCLS], f32)
                for j in range(0, NCLS, 512):
                    w = min(512, NCLS - j)
                    pcls = psE.tile([B, 512], f32, tag="pcls", bufs=2)
                    nc.tensor.matmul(pcls[:, 0:w], pool3[:],
                                     clsW_sb[:, j:j + w], start=True,
                                     stop=False)
                    nc.tensor.matmul(pcls[:, 0:w], ones1[:, 0:B],
                                     clsb_sb[:, j:j + w], start=False,
                                     stop=True)
                    nc.vector.tensor_copy(out_sb[:, j:j + w], pcls[:, 0:w])
                nc.sync.dma_start(out_t.ap(), out_sb[:])

    nc.compile()
    meta = dict(B=B, C=C, H=H, D=D, F=F, NCLS=NCLS, NCORES=NCORES,
                T=T, TS=TS, FS=FS)
    return nc, meta


def make_in_maps(x, Wg, Wu, Wd, ln_w, cls_W, cls_b, meta):
    """Host-side sharding: slices/transposes only, no arithmetic."""
    T, TS, FS = meta["T"], meta["TS"], meta["FS"]
    D = meta["D"]
    NCLS = meta["NCLS"]
    NCORES = meta["NCORES"]
    xf = np.ascontiguousarray(np.asarray(x, np.float32).reshape(T, D))
    xT = np.ascontiguousarray(xf.T)
    clsWT = np.ascontiguousarray(np.asarray(cls_W, np.float32).T)
    clsb2 = np.ascontiguousarray(np.asarray(cls_b, np.float32).reshape(1, NCLS))
    maps = []
    for k in range(NCORES):
        f0 = k * FS
        cmask = np.zeros((1, NCORES), np.float32)
        cmask[0, k] = 1.0
        maps.append({
            "xs": np.ascontiguousarray(xf[k * TS:(k + 1) * TS]),
            "xT": xT,
            "wgT": np.ascontiguousarray(np.asarray(Wg, np.float32)[f0:f0 + FS, :].T),
            "wuT": np.ascontiguousarray(np.asarray(Wu, np.float32)[f0:f0 + FS, :].T),
            "wdT": np.ascontiguousarray(np.asarray(Wd, np.float32)[:, f0:f0 + FS].T),
            "lnw": np.ascontiguousarray(np.asarray(ln_w, np.float32)[f0:f0 + FS].reshape(1, FS)),
            "clsWT": clsWT,
            "clsb": clsb2,
            "cmask": cmask,
        })
    return maps


_CACHE = {}


def kernel(x, Wg, Wu, Wd, ln_w, cls_W, cls_b):
    """Takes FULL inputs, runs the 8-core SPMD Bass kernel, returns [B, NCLS]."""
    from concourse import bass_utils

    x = np.asarray(x, np.float32)
    B, C, H, D = x.shape
    F = int(np.asarray(Wg).shape[0])
    NCLS = int(np.asarray(cls_W).shape[0])
    ln_ones = bool(np.all(np.asarray(ln_w) == 1.0))
    key = (B, C, H, D, F, NCLS, ln_ones)
    if key not in _CACHE:
        _CACHE[key] = build(B=B, C=C, H=H, D=D, F=F, NCLS=NCLS, NCORES=8,
                            ln_is_ones=ln_ones)
    nc, meta = _CACHE[key]
    in_maps = make_in_maps(x, Wg, Wu, Wd, ln_w, cls_W, cls_b, meta)
    res = bass_utils.run_bass_kernel_spmd(nc, in_maps, core_ids=list(range(8)))
    return np.asarray(res.results[0]["out"], np.float32)
